# revision 37
# baseline (speedup 1.0000x reference)
"""CloAttention Trainium2 Bass kernel.

Full inputs -> data-parallel over batch across 8 NeuronCores (4 images each)
-> full output.  All matmuls run on the PE in fp16 (1 cycle/row); the 3x3
depthwise conv runs as 9 diagonal-matmul accumulations into PSUM.

Schedule: software-pipelined per image.  Loop A runs the depthwise/gating
chain with a 2-tile skew so the PE never waits on the scalar/vector chain;
loop B runs attention + projection for image b interleaved with the qkv/gq
front-end of image b+1.  Pooling runs on the otherwise-idle GPSIMD engine;
a couple of dw-v tiles per image run as shifted multiply-adds on DVE to
shave PE work.  Weights arrive in two consolidated DMAs and dummy matmuls
warm the PE HAM clock-gate during the initial DMA wait.
"""

import numpy as np
from contextlib import ExitStack

import concourse.bacc as bacc
import concourse.bass as bass
import concourse.tile as tile
from concourse import mybir
from concourse.bass_utils import run_bass_kernel_spmd

F32 = mybir.dt.float32
F16 = mybir.dt.float16
F8 = mybir.dt.float8e4
AF = mybir.ActivationFunctionType
OP = mybir.AluOpType
DR = mybir.MatmulPerfMode.DoubleRow

# dw tap pairing for fp8 DoubleRow matmuls: 4 pairs with a constant
# address delta between the two shifted windows, plus tap 8 standalone
DW_PAIRS = ((0, 1), (3, 4), (6, 7), (2, 5))
W8_BRANCH = 4 * 256 + 128      # cols per branch in the fp8 weight block
W8_QKV = 0                     # qkv DR blocks first (DR LDW needs low offs)
W8_DW = 3 * 256                # dw branches after
W8_COLS = W8_DW + 2 * W8_BRANCH

N_CORES = 8
B_FULL = 32
B = B_FULL // N_CORES          # images per core
C = 256
H = W = 56
HW = H * W                     # 3136
PW = H + 2                     # 58 padded
NT = 7                         # pixel tiles per image
TS = HW // NT                  # 448 = 8 rows of 56
RPT = H // NT                  # 8 rows per tile
HEAD_DIM = 32
SCALER = HEAD_DIM ** -0.5
WIN = 7
HP = H // WIN                  # 8
POOL_N = HP * HP               # 64

OFFV = (1, 3, 5)               # tiles whose dw-v runs on DVE, not PE

# f16 weight block column offsets
WCOL = {}
_off = 0
for _nm, _w in (("wqkv0", 384), ("wqkv1", 384), ("dwdiag", 1152),
                ("wact1", 128), ("wact2", 128), ("wgq0", 128),
                ("wgq1", 128), ("wgkv0", 256), ("wgkv1", 256),
                ("wproj0", 256), ("wproj1", 256), ("denmask0", 128),
                ("denmask1", 128)):
    WCOL[_nm] = (_off, _off + _w)
    _off += _w
W16_COLS = _off                # 6016
W32_COLS = 15                  # dwb q,k,v | bact1 | bact2 | dwv taps 0..8 | 3.0


def _body(ctx, tc, d, n_img=B):
    nc = tc.nc

    # ---------------- persistent weights (2 consolidated DMAs) ----------
    wpool = ctx.enter_context(tc.tile_pool(name="wpool", bufs=1))

    warm_src = wpool.tile([128, 64], F16, tag="warm_src", name="warm_src")
    nc.vector.memset(warm_src, 0.0)

    wf16 = wpool.tile([128, W16_COLS], F16, tag="wf16", name="wf16")
    nc.sync.dma_start(out=wf16, in_=d["wf16"])
    wf32 = wpool.tile([128, W32_COLS], F32, tag="wf32", name="wf32")
    nc.sync.dma_start(out=wf32, in_=d["wf32"])
    wf8 = wpool.tile([128, W8_COLS], F8, tag="wf8", name="wf8")
    nc.sync.dma_start(out=wf8, in_=d["wf8"])

    def wv(name):
        a, b_ = WCOL[name]
        return wf16[:, a:b_]

    wqkv = [wv("wqkv0"), wv("wqkv1")]
    dwdiag = wv("dwdiag")
    wact1 = wv("wact1")
    wact2 = wv("wact2")
    wgq = [wv("wgq0"), wv("wgq1")]
    wgkv = [wv("wgkv0"), wv("wgkv1")]
    wproj = [wv("wproj0"), wv("wproj1")]
    denmask = [wv("denmask0"), wv("denmask1")]
    bias_q = wf32[:, 0:1]
    bias_k = wf32[:, 1:2]
    bias_v = wf32[:, 2:3]
    bact1 = wf32[:, 3:4]
    bact2 = wf32[:, 4:5]
    const3 = wf32[:, 14:15]

    def w8dr(idx):
        """fp8 DoubleRow lhsT [128, 2, 128] for qkv q(0) / k(1) / gq(2)."""
        off = W8_QKV + idx * 256
        return wf8[:, off:off + 256].rearrange("p (i m) -> p i m", i=2)

    def wv_tap(tap):
        return wf32[:, 5 + tap:6 + tap]

    def dw_lhsT(cc, tap):
        return dwdiag[:, tap * 128:(tap + 1) * 128]    # v branch only

    # padded z buffers, x2 for image parity (borders stay zero; interiors
    # rewritten per image).  q/k are fp8 (read only by the DoubleRow dw
    # matmuls); v stays fp16.
    zbufs = []
    for par in range(2):
        zs = [wpool.tile([128, PW * PW], F8 if j < 2 else F16,
                         tag=f"z{j}_{par}", name=f"z{j}_{par}")
              for j in range(3)]
        for z in zs:
            zg = z.rearrange("p (r c) -> p r c", c=PW)
            nc.vector.memset(zg[:, 0, :], 0.0)          # top border row
            nc.vector.memset(zg[:, PW - 1, :], 0.0)     # bottom border row
            nc.vector.memset(zg[:, :, 0], 0.0)          # left border col
            nc.vector.memset(zg[:, :, PW - 1], 0.0)     # right border col
        zbufs.append(zs)

    # block-diagonal gk (2 heads per matmul at K=128) and zero-padded AV
    # lhsT blocks, x2 parity; zero regions never rewritten -> memset once
    gk2 = []
    av_lhs = []
    for par in range(2):
        g = [wpool.tile([128, 128], F16, tag=f"gk2_{p}_{par}",
                        name=f"gk2_{p}_{par}") for p in range(2)]
        a = [wpool.tile([128, 128], F16, tag=f"av_{p}_{par}",
                        name=f"av_{p}_{par}") for p in range(2)]
        for tbuf in (*g, *a):
            nc.vector.memset(tbuf, 0.0)
        gk2.append(g)
        av_lhs.append(a)

    # ---------------- pools ----------------
    ps = ctx.enter_context(tc.tile_pool(name="ps", bufs=4, space="PSUM"))
    xpool = ctx.enter_context(tc.tile_pool(name="xpool", bufs=2))
    big = ctx.enter_context(tc.tile_pool(name="big", bufs=1))
    sm = ctx.enter_context(tc.tile_pool(name="sm", bufs=3))
    tiny = ctx.enter_context(tc.tile_pool(name="tiny", bufs=2))

    gq_sb2 = [big.tile([128, HW], F16, tag=f"gq_sb{i}", name=f"gq_sb{i}")
              for i in range(2)]
    exp_sb = [big.tile([128, HW], F16, tag=f"exp{p}", name=f"exp{p}")
              for p in range(2)]
    rec_rep = big.tile([128, HW], F32, tag="rec_rep")
    cat_hi2 = [big.tile([128, HW], F16, tag=f"cat_hi{i}", name=f"cat_hi{i}")
               for i in range(2)]
    cat_lo2 = [big.tile([128, HW], F16, tag=f"cat_lo{i}", name=f"cat_lo{i}")
               for i in range(2)]

    zgrid = {id(z): z.rearrange("p (r c) -> p r c", c=PW)
             for zs in zbufs for z in zs}

    def zwin(z, t, dy, dx):
        r0 = RPT * t + dy
        return zgrid[id(z)][:, r0:r0 + RPT, dx:dx + W]

    def zint(z, t):
        r0 = RPT * t + 1
        return zgrid[id(z)][:, r0:r0 + RPT, 1:1 + W]

    # PE warmup: dummy matmuls keep the HAM clock-gate busy while the
    # weight/x DMAs land, so real matmuls start at 2.4 GHz
    for wi in range(48):
        pw = ps.tile([64, 64], F32, tag="py", name="pwarm")
        nc.tensor.matmul(pw[:], warm_src[:], warm_src[:],
                         start=True, stop=True)

    # ---------------- stage helpers ----------------
    def load_x(b):
        x_sb = [xpool.tile([128, HW], F16, tag=f"x{cc}", name=f"x{cc}")
                for cc in range(2)]
        for cc in range(2):
            nc.sync.dma_start(out=x_sb[cc], in_=d["x"][b, cc])
        x8 = xpool.tile([128, 2 * HW], F8, tag="x8", name="x8")
        nc.sync.dma_start(out=x8, in_=d["x8"][b])
        # [p, 2, rows, 56]: the interleave-of-2 must be the third free dim
        # from the innermost (DoubleRow ISA requires n_elem[2]==2)
        x_sb.append(x8.rearrange("p (i r c) -> p i r c", i=2, c=W))
        return x_sb

    def qkv_tile(b, t, x_sb):
        z_q, z_k, z_v = zbufs[b % 2]
        rhs8 = x_sb[2][:, :, t * RPT:(t + 1) * RPT, :]
        for j, (z, eng) in enumerate(
                ((z_q, "act"), (z_k, "act"), (z_v, "dve"))):
            pq = ps.tile([128, TS], F32, tag="py", name="pq")
            if j < 2:
                nc.tensor.matmul(pq[:], w8dr(j), rhs8,
                                 start=True, stop=True, perf_mode=DR)
            else:
                for cc in range(2):
                    nc.tensor.matmul(
                        pq[:], wqkv[cc][:, 256:384],
                        x_sb[cc][:, t * TS:(t + 1) * TS],
                        start=(cc == 0), stop=(cc == 1))
            if eng == "act":
                nc.scalar.copy(out=zint(z, t), in_=pq[:])
            else:
                nc.vector.tensor_copy(out=zint(z, t), in_=pq[:])

    def gq_tile(b, t, x_sb):
        pg = ps.tile([128, TS], F32, tag="py", name="pg")
        for cc in range(2):
            nc.tensor.matmul(pg[:], wgq[cc][:],
                             x_sb[cc][:, t * TS:(t + 1) * TS],
                             start=(cc == 0), stop=(cc == 1))
        nc.scalar.copy(out=gq_sb2[b % 2][:, t * TS:(t + 1) * TS],
                       in_=pg[:])

    def pool_reduce(x_sb):
        """7x7 window sums; issued early so the results have slack."""
        pooled = []
        for cc in range(2):
            pr1 = sm.tile([128, H * HP], F32, tag="pr1", name="pr1")
            nc.vector.tensor_reduce(
                out=pr1.rearrange("p (y g) -> p y g", g=HP),
                in_=x_sb[cc].rearrange("p (y g x) -> p y g x", y=H, g=HP),
                axis=mybir.AxisListType.X, op=OP.add)
            po = tiny.tile([128, POOL_N], F16, tag="po", name="po")
            with nc.allow_low_precision(reason="pool sums fit fp16"):
                nc.vector.tensor_reduce(
                    out=po.rearrange("p (a b) -> p a b", a=HP),
                    in_=pr1.rearrange("p (hp dy wp) -> p hp wp dy",
                                      hp=HP, dy=WIN),
                    axis=mybir.AxisListType.X, op=OP.add)
            pooled.append(po)
        return pooled

    def pool_finish(b, pooled):
        """global-kv matmuls + lhsT packing for image b's attention."""
        par = b % 2
        pgk = ps.tile([128, POOL_N], F32, tag="py", name="pgk")
        for cc in range(2):
            nc.tensor.matmul(pgk[:], wgkv[cc][:, 0:128], pooled[cc][:],
                             start=(cc == 0), stop=(cc == 1))
        for p in range(2):
            for hl in range(2):
                h = 2 * p + hl
                nc.scalar.copy(
                    out=gk2[par][p][32 * h:32 * h + 32,
                                    64 * hl:64 * hl + 64],
                    in_=pgk[32 * h:32 * h + 32, :])
        pgv = ps.tile([POOL_N, 128], F32, tag="py", name="pgv")
        for cc in range(2):
            nc.tensor.matmul(pgv[:], pooled[cc][:], wgkv[cc][:, 128:256],
                             start=(cc == 0), stop=(cc == 1))
        gvT = tiny.tile([POOL_N, 128], F16, tag="gvT", name="gvT")
        nc.scalar.copy(out=gvT[:], in_=pgv[:])
        av0, av1 = av_lhs[par]
        nc.vector.tensor_copy(out=av0[0:64, 0:32], in_=gvT[:, 0:32])
        nc.sync.dma_start(out=av0[64:128, 32:64], in_=gvT[:, 32:64])
        nc.vector.tensor_copy(out=av1[0:64, 64:96], in_=gvT[:, 64:96])
        nc.sync.dma_start(out=av1[64:128, 96:128], in_=gvT[:, 96:128])

    def dw_mm(z, cc, t, psname):
        p = ps.tile([128, TS], F32, tag="px", name=psname)
        for tap in range(9):
            dy, dx = divmod(tap, 3)
            nc.tensor.matmul(p[:], dw_lhsT(cc, tap), zwin(z, t, dy, dx),
                             start=(tap == 0), stop=(tap == 8))
        return p

    def dw_mm8(z8, br, t, psname):
        """dw conv via 4 fp8 DoubleRow pair-matmuls + 1 plain fp8 matmul."""
        p = ps.tile([128, TS], F32, tag="px", name=psname)
        zg = zgrid[id(z8)]
        for pr, (tapA, tapB) in enumerate(DW_PAIRS):
            dyA, dxA = divmod(tapA, 3)
            dyB, dxB = divmod(tapB, 3)
            delta = (dyB - dyA) * PW + (dxB - dxA)
            w = zg[:, RPT * t + dyA:RPT * t + dyA + RPT, dxA:dxA + W]
            pa = list(w.ap)
            rhs = bass.AP(w.tensor, w.offset,
                          [pa[0], [delta, 2], pa[1], pa[2]])
            lhsT = wf8[:, W8_DW + br * W8_BRANCH + pr * 256:
                       W8_DW + br * W8_BRANCH + (pr + 1) * 256]
            nc.tensor.matmul(p[:], lhsT.rearrange("p (i m) -> p i m", i=2),
                             rhs, start=(pr == 0), stop=False,
                             perf_mode=mybir.MatmulPerfMode.DoubleRow)
        nc.tensor.matmul(p[:],
                         wf8[:, W8_DW + br * W8_BRANCH + 1024:
                             W8_DW + br * W8_BRANCH + 1152],
                         zwin(z8, t, 2, 2), start=False, stop=True)
        return p

    def dwv_vector(z_v, t):
        """dw-v for one tile as 9 shifted multiply-adds on DVE; returns
        the accumulated (dwv + bias_v) tile in fp16."""
        acc = sm.tile([128, TS], F16, tag="accv", name="accv")
        with nc.allow_low_precision(reason="dwv fits fp16"):
            nc.vector.tensor_scalar(
                out=acc[:], in0=zwin(z_v, t, 0, 0), scalar1=wv_tap(0),
                scalar2=bias_v, op0=OP.mult, op1=OP.add)
            for tap in range(1, 9):
                dy, dx = divmod(tap, 3)
                nacc = sm.tile([128, TS], F16, tag="accv", name="accv")
                nc.vector.scalar_tensor_tensor(
                    out=nacc[:], in0=zwin(z_v, t, dy, dx),
                    scalar=wv_tap(tap), in1=acc[:],
                    op0=OP.mult, op1=OP.add)
                acc = nacc
        return acc

    # ---------------- pipelined loops ----------------
    def loop_a(b):
        """dwconv + gating chain, 2-tile skew."""
        z_q, z_k, z_v = zbufs[b % 2]
        cat_hi = cat_hi2[b % 2]
        qk_t = {}
        hs = {}
        for i in range(NT + 2):
            if i < NT:
                t = i
                pdq = dw_mm8(z_q, 0, t, "pdq")
                q_t = sm.tile([128, TS], F16, tag="q_t", name="q_t")
                nc.scalar.activation(out=q_t[:], in_=pdq[:],
                                     func=AF.Identity, bias=bias_q)
                pdk = dw_mm8(z_k, 1, t, "pdk")
                qk = sm.tile([128, TS], F16, tag="qk_t", name="qk_t")
                with nc.allow_low_precision(reason="qk product fits fp16"):
                    nc.vector.scalar_tensor_tensor(
                        out=qk[:], in0=pdk[:], scalar=bias_k, in1=q_t[:],
                        op0=OP.add, op1=OP.mult)
                qk_t[t] = qk
            if 1 <= i <= NT:
                t = i - 1
                pa1 = ps.tile([128, TS], F32, tag="py", name="pa1")
                nc.tensor.matmul(pa1[:], wact1[:], qk_t[t][:],
                                 start=True, stop=True)
                t_a = sm.tile([128, TS], F16, tag="t_a", name="t_a")
                nc.scalar.activation(out=t_a[:], in_=pa1[:],
                                     func=AF.Identity, bias=bact1)
                u_t = sm.tile([128, TS], F16, tag="u_t", name="u_t")
                nc.scalar.activation(out=u_t[:], in_=t_a[:],
                                     func=AF.Relu, bias=const3)
                h_t = sm.tile([128, TS], F16, tag="hs_t", name="hs_t")
                with nc.allow_low_precision(reason="hardswish fits fp16"):
                    nc.vector.scalar_tensor_tensor(
                        out=h_t[:], in0=u_t[:], scalar=6.0, in1=t_a[:],
                        op0=OP.min, op1=OP.mult)
                hs[t] = h_t
            if 2 <= i:
                t = i - 2
                sl = slice(t * TS, (t + 1) * TS)
                pa2 = ps.tile([128, TS], F32, tag="py", name="pa2")
                nc.tensor.matmul(pa2[:], wact2[:], hs[t][:],
                                 start=True, stop=True)
                g_t = sm.tile([128, TS], F16, tag="g_t", name="g_t")
                nc.scalar.activation(out=g_t[:], in_=pa2[:], func=AF.Tanh,
                                     bias=bact2)
                if t in OFFV:
                    acc = dwv_vector(z_v, t)
                    with nc.allow_low_precision(reason="gated out fp16"):
                        nc.vector.scalar_tensor_tensor(
                            out=cat_hi[:, sl], in0=acc[:], scalar=1.0,
                            in1=g_t[:], op0=OP.mult, op1=OP.mult)
                else:
                    pdv = dw_mm(z_v, 2, t, "pdv")
                    with nc.allow_low_precision(reason="gated out fp16"):
                        nc.vector.scalar_tensor_tensor(
                            out=cat_hi[:, sl], in0=pdv[:], scalar=bias_v,
                            in1=g_t[:], op0=OP.add, op1=OP.mult)

    def scores_stage(par, t, gq_sb):
        sl = slice(t * TS, (t + 1) * TS)
        for p in range(2):
            pat = ps.tile([128, TS], F32, tag="px", name="pat")
            nc.tensor.matmul(pat[:], gk2[par][p][:], gq_sb[:, sl],
                             start=True, stop=True)
            nc.scalar.activation(out=exp_sb[p][:, sl], in_=pat[:],
                                 func=AF.Exp, scale=float(SCALER))

    def den_stage(t):
        sl = slice(t * TS, (t + 1) * TS)
        pden = ps.tile([128, TS], F32, tag="px", name="pden")
        for p in range(2):
            nc.tensor.matmul(pden[:], denmask[p][:], exp_sb[p][:, sl],
                             start=(p == 0), stop=(p == 1))
        nc.vector.reciprocal_approx_fast(out=rec_rep[:, sl], in_=pden[:])

    def av_stage(t, par, cat_lo):
        sl = slice(t * TS, (t + 1) * TS)
        av0, av1 = av_lhs[par]
        pav = ps.tile([128, TS], F32, tag="px", name="pav")
        nc.tensor.matmul(pav[:], av0[:], exp_sb[0][:, sl],
                         start=True, stop=False)
        nc.tensor.matmul(pav[:], av1[:], exp_sb[1][:, sl],
                         start=False, stop=True)
        with nc.allow_low_precision(reason="attn out fits fp16"):
            nc.vector.scalar_tensor_tensor(
                out=cat_lo[:, sl], in0=pav[:], scalar=1.0,
                in1=rec_rep[:, sl], op0=OP.mult, op1=OP.mult)

    def proj_stage(b, t, cat_hi, cat_lo):
        sl = slice(t * TS, (t + 1) * TS)
        for m in range(2):
            pp = ps.tile([128, TS], F32, tag="py", name="pp")
            nc.tensor.matmul(pp[:], wproj[0][:, m * 128:(m + 1) * 128],
                             cat_hi[:, sl], start=True, stop=False)
            nc.tensor.matmul(pp[:], wproj[1][:, m * 128:(m + 1) * 128],
                             cat_lo[:, sl], start=False, stop=True)
            o_t = sm.tile([128, TS], F16, tag=f"o_t{m}", name=f"o_t{m}")
            nc.scalar.copy(out=o_t[:], in_=pp[:])
            nc.sync.dma_start(out=d["out"][b, m, :, sl], in_=o_t)

    def pool_closures(x_sb):
        """4 GPSIMD-free pooling ops as closures, sprinkled across stages."""
        ops = []
        pooled = []
        for cc in range(2):
            pr1 = sm.tile([128, H * HP], F32, tag="pr1", name="pr1")
            po = tiny.tile([128, POOL_N], F16, tag="po", name="po")
            pooled.append(po)

            def st1(cc=cc, pr1=pr1):
                nc.vector.tensor_reduce(
                    out=pr1.rearrange("p (y g) -> p y g", g=HP),
                    in_=x_sb[cc].rearrange("p (y g x) -> p y g x",
                                           y=H, g=HP),
                    axis=mybir.AxisListType.X, op=OP.add)

            def st2(pr1=pr1, po=po):
                with nc.allow_low_precision(reason="pool sums fit fp16"):
                    nc.vector.tensor_reduce(
                        out=po.rearrange("p (a b) -> p a b", a=HP),
                        in_=pr1.rearrange("p (hp dy wp) -> p hp wp dy",
                                          hp=HP, dy=WIN),
                        axis=mybir.AxisListType.X, op=OP.add)
            ops.append(st1)
            ops.append(st2)
        return ops, pooled

    def unified(b, x_next):
        """attention + projection for image b fully interleaved with the
        qkv/gq front-end and dw/gating chain of image b+1."""
        par = b % 2
        npar = 1 - par
        cat_hi = cat_hi2[par]
        cat_lo = cat_lo2[par]
        gq_sb = gq_sb2[par]
        nz_q, nz_k, nz_v = zbufs[npar]
        ncat_hi = cat_hi2[npar]
        qk_t = {}
        hs = {}
        have = x_next is not None
        if have:
            pops, pooled = pool_closures(x_next)
        for i in range(NT + 5):
            if have and i < NT:
                qkv_tile(b + 1, i, x_next)
                gq_tile(b + 1, i, x_next)
            if i < NT:
                scores_stage(par, i, gq_sb)
            if have and 2 <= i < NT + 2:
                t = i - 2
                pdq = dw_mm8(nz_q, 0, t, "pdq")
                q_t = sm.tile([128, TS], F16, tag="q_t", name="q_t")
                nc.scalar.activation(out=q_t[:], in_=pdq[:],
                                     func=AF.Identity, bias=bias_q)
                pdk = dw_mm8(nz_k, 1, t, "pdk")
                qk = sm.tile([128, TS], F16, tag="qk_t", name="qk_t")
                with nc.allow_low_precision(reason="qk product fits fp16"):
                    nc.vector.scalar_tensor_tensor(
                        out=qk[:], in0=pdk[:], scalar=bias_k, in1=q_t[:],
                        op0=OP.add, op1=OP.mult)
                qk_t[t] = qk
            if 1 <= i <= NT:
                den_stage(i - 1)
            if have and 3 <= i < NT + 3:
                t = i - 3
                pa1 = ps.tile([128, TS], F32, tag="py", name="pa1")
                nc.tensor.matmul(pa1[:], wact1[:], qk_t[t][:],
                                 start=True, stop=True)
                t_a = sm.tile([128, TS], F16, tag="t_a", name="t_a")
                nc.scalar.activation(out=t_a[:], in_=pa1[:],
                                     func=AF.Identity, bias=bact1)
                u_t = sm.tile([128, TS], F16, tag="u_t", name="u_t")
                nc.scalar.activation(out=u_t[:], in_=t_a[:],
                                     func=AF.Relu, bias=const3)
                h_t = sm.tile([128, TS], F16, tag="hs_t", name="hs_t")
                with nc.allow_low_precision(reason="hardswish fits fp16"):
                    nc.vector.scalar_tensor_tensor(
                        out=h_t[:], in0=u_t[:], scalar=6.0, in1=t_a[:],
                        op0=OP.min, op1=OP.mult)
                hs[t] = h_t
            if 2 <= i <= NT + 1:
                av_stage(i - 2, par, cat_lo)
            if have and 4 <= i < NT + 4:
                t = i - 4
                sl = slice(t * TS, (t + 1) * TS)
                pa2 = ps.tile([128, TS], F32, tag="py", name="pa2")
                nc.tensor.matmul(pa2[:], wact2[:], hs[t][:],
                                 start=True, stop=True)
                g_t = sm.tile([128, TS], F16, tag="g_t", name="g_t")
                nc.scalar.activation(out=g_t[:], in_=pa2[:], func=AF.Tanh,
                                     bias=bact2)
                if t in OFFV:
                    acc = dwv_vector(nz_v, t)
                    with nc.allow_low_precision(reason="gated out fp16"):
                        nc.vector.scalar_tensor_tensor(
                            out=ncat_hi[:, sl], in0=acc[:], scalar=1.0,
                            in1=g_t[:], op0=OP.mult, op1=OP.mult)
                else:
                    pdv = dw_mm(nz_v, 2, t, "pdv")
                    with nc.allow_low_precision(reason="gated out fp16"):
                        nc.vector.scalar_tensor_tensor(
                            out=ncat_hi[:, sl], in0=pdv[:], scalar=bias_v,
                            in1=g_t[:], op0=OP.add, op1=OP.mult)
            if 3 <= i <= NT + 2:
                proj_stage(b, i - 3, cat_hi, cat_lo)
            if have and 2 <= i <= 5:
                pops[i - 2]()
        if have:
            pool_finish(b + 1, pooled)

    # ---------------- program ----------------
    x_cur = load_x(0)
    pooled = pool_reduce(x_cur)
    for t in range(NT):
        qkv_tile(0, t, x_cur)
        gq_tile(0, t, x_cur)
    pool_finish(0, pooled)
    loop_a(0)

    for b in range(n_img):
        x_next = load_x(b + 1) if b + 1 < n_img else None
        unified(b, x_next)


def _build(n_img=B):
    nc = bacc.Bacc("TRN2", target_bir_lowering=False, debug=False,
                   num_devices=N_CORES)
    dt = nc.dram_tensor
    d = {
        "x": dt("x", [B, 2, 128, HW], F16, kind="ExternalInput").ap(),
        "x8": dt("x8", [B, 128, 2 * HW], F8, kind="ExternalInput").ap(),
        "wf16": dt("wf16", [128, W16_COLS], F16, kind="ExternalInput").ap(),
        "wf32": dt("wf32", [128, W32_COLS], F32, kind="ExternalInput").ap(),
        "wf8": dt("wf8", [128, W8_COLS], F8, kind="ExternalInput").ap(),
        "out": dt("out", [B, 2, 128, HW], F16, kind="ExternalOutput").ap(),
    }
    with tile.TileContext(nc) as tc, ExitStack() as ctx:
        _body(ctx, tc, d, n_img=n_img)
    nc.compile()
    return nc


_NC = None


def _prep_weights(qkv_w, dw_w, dw_b, act1_w, act1_b, act2_w, act2_b,
                  gq_w, gkv_w, proj_w):
    f32 = np.float32
    f16 = np.float16
    sc = np.float32(HEAD_DIM ** -0.5)

    wqkv = qkv_w.T.reshape(2, 128, 384).astype(f16)
    taps = dw_w.reshape(384, 9)            # [c, tap]
    idx = np.arange(128)
    # f16 diag blocks for the v branch only
    dwd = np.zeros((9, 128, 128), dtype=f16)
    for tp in range(9):
        dwd[tp, idx, idx] = taps[256:384, tp]
    dwdiag = dwd.transpose(1, 0, 2).reshape(128, 9 * 128)
    # fp8 DoubleRow pair blocks for q and k branches
    f8 = np.dtype(np.float32)  # placeholder; real cast below
    import ml_dtypes
    e4 = ml_dtypes.float8_e4m3
    wf8 = np.zeros((128, W8_COLS), dtype=e4)
    for br in range(2):
        tb = taps[128 * br:128 * (br + 1)]
        for pr, (ta_, tb_) in enumerate(DW_PAIRS):
            blk = np.zeros((128, 2, 128), np.float32)
            blk[idx, 0, idx] = tb[:, ta_]
            blk[idx, 1, idx] = tb[:, tb_]
            wf8[:, W8_DW + br * W8_BRANCH + pr * 256:
                W8_DW + br * W8_BRANCH + (pr + 1) * 256] = (
                blk.reshape(128, 256).astype(e4))
        t8 = np.zeros((128, 128), np.float32)
        t8[idx, idx] = tb[:, 8]
        wf8[:, W8_DW + br * W8_BRANCH + 1024:
            W8_DW + br * W8_BRANCH + 1152] = t8.astype(e4)
    for bi, wsrc in enumerate((qkv_w[0:128], qkv_w[128:256], gq_w)):
        blk = (wsrc.T.reshape(2, 128, 128).transpose(1, 0, 2)
               .reshape(128, 256))
        wf8[:, W8_QKV + bi * 256:W8_QKV + (bi + 1) * 256] = blk.astype(e4)
    wact1 = (act1_w * sc).T.astype(f16)
    wact2 = (act2_w / 6.0).T.astype(f16)
    wgq = gq_w.T.reshape(2, 128, 128).astype(f16)
    wgkv = (gkv_w / 49.0).T.reshape(2, 128, 256).astype(f16)
    wproj = proj_w.T.reshape(2, 128, 256).astype(f16)
    dm = np.zeros((2, 128, 128), dtype=f16)
    for p in range(2):
        for hl in range(2):
            head = 2 * p + hl
            dm[p, 64 * hl:64 * hl + 64, 32 * head:32 * head + 32] = 1.0

    blocks = {"wqkv0": wqkv[0], "wqkv1": wqkv[1], "dwdiag": dwdiag,
              "wact1": wact1, "wact2": wact2, "wgq0": wgq[0],
              "wgq1": wgq[1], "wgkv0": wgkv[0], "wgkv1": wgkv[1],
              "wproj0": wproj[0], "wproj1": wproj[1],
              "denmask0": dm[0], "denmask1": dm[1]}
    wf16 = np.zeros((128, W16_COLS), dtype=f16)
    for nm, (a, b_) in WCOL.items():
        wf16[:, a:b_] = blocks[nm]

    wf32 = np.zeros((128, W32_COLS), dtype=f32)
    wf32[:, 0:3] = dw_b.reshape(3, 128).T
    wf32[:, 3] = act1_b.astype(f32)
    wf32[:, 4] = act2_b.astype(f32)
    wf32[:, 5:14] = taps[256:384].astype(f32)   # dw-v taps for DVE path
    wf32[:, 14] = 3.0

    return {"wf16": np.ascontiguousarray(wf16),
            "wf32": np.ascontiguousarray(wf32),
            "wf8": np.ascontiguousarray(wf8)}


def _make_in_maps(inputs):
    w = _prep_weights(
        inputs["qkv_w"], inputs["dw_w"], inputs["dw_b"],
        inputs["act1_w"], inputs["act1_b"], inputs["act2_w"],
        inputs["act2_b"], inputs["gq_w"], inputs["gkv_w"],
        inputs["proj_w"])
    x = inputs["x"]
    in_maps = []
    for core in range(N_CORES):
        m = dict(w)
        xc = x[core * B:(core + 1) * B]
        m["x"] = np.ascontiguousarray(
            xc.reshape(B, 2, 128, HW).astype(np.float16))
        import ml_dtypes
        m["x8"] = np.ascontiguousarray(
            xc.reshape(B, 2, 128, HW).transpose(0, 2, 1, 3)
            .reshape(B, 128, 2 * HW).astype(ml_dtypes.float8_e4m3))
        in_maps.append(m)
    return in_maps


def kernel(**inputs):
    global _NC
    if _NC is None:
        _NC = _build()
    in_maps = _make_in_maps(inputs)
    res = run_bass_kernel_spmd(_NC, in_maps, core_ids=list(range(N_CORES)))
    out = np.concatenate([r["out"] for r in res.results], axis=0)
    return out.reshape(B_FULL, C, H, W).astype(np.float32)


# revision 38
# speedup vs baseline: 1.0097x; 1.0097x over previous
"""CloAttention Trainium2 Bass kernel.

Full inputs -> data-parallel over batch across 8 NeuronCores (4 images each)
-> full output.  All matmuls run on the PE in fp16 (1 cycle/row); the 3x3
depthwise conv runs as 9 diagonal-matmul accumulations into PSUM.

Schedule: software-pipelined per image.  Loop A runs the depthwise/gating
chain with a 2-tile skew so the PE never waits on the scalar/vector chain;
loop B runs attention + projection for image b interleaved with the qkv/gq
front-end of image b+1.  Pooling runs on the otherwise-idle GPSIMD engine;
a couple of dw-v tiles per image run as shifted multiply-adds on DVE to
shave PE work.  Weights arrive in two consolidated DMAs and dummy matmuls
warm the PE HAM clock-gate during the initial DMA wait.
"""

import numpy as np
from contextlib import ExitStack

import concourse.bacc as bacc
import concourse.bass as bass
import concourse.tile as tile
from concourse import mybir
from concourse.bass_utils import run_bass_kernel_spmd

F32 = mybir.dt.float32
F16 = mybir.dt.float16
F8 = mybir.dt.float8e4
AF = mybir.ActivationFunctionType
OP = mybir.AluOpType
DR = mybir.MatmulPerfMode.DoubleRow

# dw tap pairing for fp8 DoubleRow matmuls: 4 pairs with a constant
# address delta between the two shifted windows, plus tap 8 standalone
DW_PAIRS = ((0, 1), (3, 4), (6, 7), (2, 5))
W8_BRANCH = 4 * 256 + 128      # cols per branch in the fp8 weight block
W8_QKV = 0                     # qkv DR blocks first (DR LDW needs low offs)
W8_DW = 3 * 256                # dw branches after
W8_COLS = W8_DW + 2 * W8_BRANCH

N_CORES = 8
B_FULL = 32
B = B_FULL // N_CORES          # images per core
C = 256
H = W = 56
HW = H * W                     # 3136
PW = H + 2                     # 58 padded
NT = 7                         # pixel tiles per image
TS = HW // NT                  # 448 = 8 rows of 56
RPT = H // NT                  # 8 rows per tile
HEAD_DIM = 32
SCALER = HEAD_DIM ** -0.5
WIN = 7
HP = H // WIN                  # 8
POOL_N = HP * HP               # 64

OFFV = (1, 4)                  # tiles whose dw-v runs on DVE, not PE

# f16 weight block column offsets
WCOL = {}
_off = 0
for _nm, _w in (("wqkv0", 384), ("wqkv1", 384), ("dwdiag", 1152),
                ("wact1", 128), ("wact2", 128), ("wgq0", 128),
                ("wgq1", 128), ("wgkv0", 256), ("wgkv1", 256),
                ("wproj0", 256), ("wproj1", 256), ("denmask0", 128),
                ("denmask1", 128)):
    WCOL[_nm] = (_off, _off + _w)
    _off += _w
W16_COLS = _off                # 6016
W32_COLS = 15                  # dwb q,k,v | bact1 | bact2 | dwv taps 0..8 | 3.0


def _body(ctx, tc, d, n_img=B):
    nc = tc.nc

    # ---------------- persistent weights (2 consolidated DMAs) ----------
    wpool = ctx.enter_context(tc.tile_pool(name="wpool", bufs=1))

    warm_src = wpool.tile([128, 64], F16, tag="warm_src", name="warm_src")
    nc.vector.memset(warm_src, 0.0)

    wf16 = wpool.tile([128, W16_COLS], F16, tag="wf16", name="wf16")
    nc.sync.dma_start(out=wf16, in_=d["wf16"])
    wf32 = wpool.tile([128, W32_COLS], F32, tag="wf32", name="wf32")
    nc.sync.dma_start(out=wf32, in_=d["wf32"])
    wf8 = wpool.tile([128, W8_COLS], F8, tag="wf8", name="wf8")
    nc.sync.dma_start(out=wf8, in_=d["wf8"])

    def wv(name):
        a, b_ = WCOL[name]
        return wf16[:, a:b_]

    wqkv = [wv("wqkv0"), wv("wqkv1")]
    dwdiag = wv("dwdiag")
    wact1 = wv("wact1")
    wact2 = wv("wact2")
    wgq = [wv("wgq0"), wv("wgq1")]
    wgkv = [wv("wgkv0"), wv("wgkv1")]
    wproj = [wv("wproj0"), wv("wproj1")]
    denmask = [wv("denmask0"), wv("denmask1")]
    bias_q = wf32[:, 0:1]
    bias_k = wf32[:, 1:2]
    bias_v = wf32[:, 2:3]
    bact1 = wf32[:, 3:4]
    bact2 = wf32[:, 4:5]
    const3 = wf32[:, 14:15]

    def w8dr(idx):
        """fp8 DoubleRow lhsT [128, 2, 128] for qkv q(0) / k(1) / gq(2)."""
        off = W8_QKV + idx * 256
        return wf8[:, off:off + 256].rearrange("p (i m) -> p i m", i=2)

    def wv_tap(tap):
        return wf32[:, 5 + tap:6 + tap]

    def dw_lhsT(cc, tap):
        return dwdiag[:, tap * 128:(tap + 1) * 128]    # v branch only

    # padded z buffers, x2 for image parity (borders stay zero; interiors
    # rewritten per image).  q/k are fp8 (read only by the DoubleRow dw
    # matmuls); v stays fp16.
    zbufs = []
    for par in range(2):
        zs = [wpool.tile([128, PW * PW], F8 if j < 2 else F16,
                         tag=f"z{j}_{par}", name=f"z{j}_{par}")
              for j in range(3)]
        for z in zs:
            zg = z.rearrange("p (r c) -> p r c", c=PW)
            nc.vector.memset(zg[:, 0, :], 0.0)          # top border row
            nc.vector.memset(zg[:, PW - 1, :], 0.0)     # bottom border row
            nc.vector.memset(zg[:, :, 0], 0.0)          # left border col
            nc.vector.memset(zg[:, :, PW - 1], 0.0)     # right border col
        zbufs.append(zs)

    # block-diagonal gk (2 heads per matmul at K=128) and zero-padded AV
    # lhsT blocks, x2 parity; zero regions never rewritten -> memset once
    gk2 = []
    av_lhs = []
    for par in range(2):
        g = [wpool.tile([128, 128], F16, tag=f"gk2_{p}_{par}",
                        name=f"gk2_{p}_{par}") for p in range(2)]
        a = [wpool.tile([128, 128], F16, tag=f"av_{p}_{par}",
                        name=f"av_{p}_{par}") for p in range(2)]
        for tbuf in (*g, *a):
            nc.vector.memset(tbuf, 0.0)
        gk2.append(g)
        av_lhs.append(a)

    # ---------------- pools ----------------
    ps = ctx.enter_context(tc.tile_pool(name="ps", bufs=4, space="PSUM"))
    xpool = ctx.enter_context(tc.tile_pool(name="xpool", bufs=2))
    big = ctx.enter_context(tc.tile_pool(name="big", bufs=1))
    sm = ctx.enter_context(tc.tile_pool(name="sm", bufs=3))
    tiny = ctx.enter_context(tc.tile_pool(name="tiny", bufs=2))

    gq_sb2 = [big.tile([128, HW], F16, tag=f"gq_sb{i}", name=f"gq_sb{i}")
              for i in range(2)]
    exp_sb = [big.tile([128, HW], F16, tag=f"exp{p}", name=f"exp{p}")
              for p in range(2)]
    rec_rep = big.tile([128, HW], F32, tag="rec_rep")
    cat_hi2 = [big.tile([128, HW], F16, tag=f"cat_hi{i}", name=f"cat_hi{i}")
               for i in range(2)]
    cat_lo2 = [big.tile([128, HW], F16, tag=f"cat_lo{i}", name=f"cat_lo{i}")
               for i in range(2)]

    zgrid = {id(z): z.rearrange("p (r c) -> p r c", c=PW)
             for zs in zbufs for z in zs}

    def zwin(z, t, dy, dx):
        r0 = RPT * t + dy
        return zgrid[id(z)][:, r0:r0 + RPT, dx:dx + W]

    def zint(z, t):
        r0 = RPT * t + 1
        return zgrid[id(z)][:, r0:r0 + RPT, 1:1 + W]

    # PE warmup: dummy matmuls keep the HAM clock-gate busy while the
    # weight/x DMAs land, so real matmuls start at 2.4 GHz
    for wi in range(48):
        pw = ps.tile([64, 64], F32, tag="py", name="pwarm")
        nc.tensor.matmul(pw[:], warm_src[:], warm_src[:],
                         start=True, stop=True)

    # ---------------- stage helpers ----------------
    def load_x(b):
        x_sb = [xpool.tile([128, HW], F16, tag=f"x{cc}", name=f"x{cc}")
                for cc in range(2)]
        for cc in range(2):
            nc.sync.dma_start(out=x_sb[cc], in_=d["x"][b, cc])
        x8 = xpool.tile([128, 2 * HW], F8, tag="x8", name="x8")
        nc.sync.dma_start(out=x8, in_=d["x8"][b])
        # [p, 2, rows, 56]: the interleave-of-2 must be the third free dim
        # from the innermost (DoubleRow ISA requires n_elem[2]==2)
        x_sb.append(x8.rearrange("p (i r c) -> p i r c", i=2, c=W))
        return x_sb

    def qkv_tile(b, t, x_sb):
        z_q, z_k, z_v = zbufs[b % 2]
        rhs8 = x_sb[2][:, :, t * RPT:(t + 1) * RPT, :]
        for j, (z, eng) in enumerate(
                ((z_q, "act"), (z_k, "act"), (z_v, "dve"))):
            pq = ps.tile([128, TS], F32, tag="py", name="pq")
            if j < 2:
                nc.tensor.matmul(pq[:], w8dr(j), rhs8,
                                 start=True, stop=True, perf_mode=DR)
            else:
                for cc in range(2):
                    nc.tensor.matmul(
                        pq[:], wqkv[cc][:, 256:384],
                        x_sb[cc][:, t * TS:(t + 1) * TS],
                        start=(cc == 0), stop=(cc == 1))
            if eng == "act":
                nc.scalar.copy(out=zint(z, t), in_=pq[:])
            else:
                nc.vector.tensor_copy(out=zint(z, t), in_=pq[:])

    def gq_tile(b, t, x_sb):
        pg = ps.tile([128, TS], F32, tag="py", name="pg")
        for cc in range(2):
            nc.tensor.matmul(pg[:], wgq[cc][:],
                             x_sb[cc][:, t * TS:(t + 1) * TS],
                             start=(cc == 0), stop=(cc == 1))
        nc.scalar.copy(out=gq_sb2[b % 2][:, t * TS:(t + 1) * TS],
                       in_=pg[:])

    def pool_reduce(x_sb):
        """7x7 window sums; issued early so the results have slack."""
        pooled = []
        for cc in range(2):
            pr1 = sm.tile([128, H * HP], F32, tag="pr1", name="pr1")
            nc.vector.tensor_reduce(
                out=pr1.rearrange("p (y g) -> p y g", g=HP),
                in_=x_sb[cc].rearrange("p (y g x) -> p y g x", y=H, g=HP),
                axis=mybir.AxisListType.X, op=OP.add)
            po = tiny.tile([128, POOL_N], F16, tag="po", name="po")
            with nc.allow_low_precision(reason="pool sums fit fp16"):
                nc.vector.tensor_reduce(
                    out=po.rearrange("p (a b) -> p a b", a=HP),
                    in_=pr1.rearrange("p (hp dy wp) -> p hp wp dy",
                                      hp=HP, dy=WIN),
                    axis=mybir.AxisListType.X, op=OP.add)
            pooled.append(po)
        return pooled

    def pool_finish(b, pooled):
        """global-kv matmuls + lhsT packing for image b's attention."""
        par = b % 2
        pgk = ps.tile([128, POOL_N], F32, tag="py", name="pgk")
        for cc in range(2):
            nc.tensor.matmul(pgk[:], wgkv[cc][:, 0:128], pooled[cc][:],
                             start=(cc == 0), stop=(cc == 1))
        for p in range(2):
            for hl in range(2):
                h = 2 * p + hl
                nc.scalar.copy(
                    out=gk2[par][p][32 * h:32 * h + 32,
                                    64 * hl:64 * hl + 64],
                    in_=pgk[32 * h:32 * h + 32, :])
        pgv = ps.tile([POOL_N, 128], F32, tag="py", name="pgv")
        for cc in range(2):
            nc.tensor.matmul(pgv[:], pooled[cc][:], wgkv[cc][:, 128:256],
                             start=(cc == 0), stop=(cc == 1))
        gvT = tiny.tile([POOL_N, 128], F16, tag="gvT", name="gvT")
        nc.scalar.copy(out=gvT[:], in_=pgv[:])
        av0, av1 = av_lhs[par]
        nc.vector.tensor_copy(out=av0[0:64, 0:32], in_=gvT[:, 0:32])
        nc.sync.dma_start(out=av0[64:128, 32:64], in_=gvT[:, 32:64])
        nc.vector.tensor_copy(out=av1[0:64, 64:96], in_=gvT[:, 64:96])
        nc.sync.dma_start(out=av1[64:128, 96:128], in_=gvT[:, 96:128])

    def dw_mm(z, cc, t, psname):
        p = ps.tile([128, TS], F32, tag="px", name=psname)
        for tap in range(9):
            dy, dx = divmod(tap, 3)
            nc.tensor.matmul(p[:], dw_lhsT(cc, tap), zwin(z, t, dy, dx),
                             start=(tap == 0), stop=(tap == 8))
        return p

    def dw_mm8(z8, br, t, psname):
        """dw conv via 4 fp8 DoubleRow pair-matmuls + 1 plain fp8 matmul."""
        p = ps.tile([128, TS], F32, tag="px", name=psname)
        zg = zgrid[id(z8)]
        for pr, (tapA, tapB) in enumerate(DW_PAIRS):
            dyA, dxA = divmod(tapA, 3)
            dyB, dxB = divmod(tapB, 3)
            delta = (dyB - dyA) * PW + (dxB - dxA)
            w = zg[:, RPT * t + dyA:RPT * t + dyA + RPT, dxA:dxA + W]
            pa = list(w.ap)
            rhs = bass.AP(w.tensor, w.offset,
                          [pa[0], [delta, 2], pa[1], pa[2]])
            lhsT = wf8[:, W8_DW + br * W8_BRANCH + pr * 256:
                       W8_DW + br * W8_BRANCH + (pr + 1) * 256]
            nc.tensor.matmul(p[:], lhsT.rearrange("p (i m) -> p i m", i=2),
                             rhs, start=(pr == 0), stop=False,
                             perf_mode=mybir.MatmulPerfMode.DoubleRow)
        nc.tensor.matmul(p[:],
                         wf8[:, W8_DW + br * W8_BRANCH + 1024:
                             W8_DW + br * W8_BRANCH + 1152],
                         zwin(z8, t, 2, 2), start=False, stop=True)
        return p

    def dwv_vector(z_v, t):
        """dw-v for one tile as 9 shifted multiply-adds on DVE; returns
        the accumulated (dwv + bias_v) tile in fp16."""
        acc = sm.tile([128, TS], F16, tag="accv", name="accv")
        with nc.allow_low_precision(reason="dwv fits fp16"):
            nc.vector.tensor_scalar(
                out=acc[:], in0=zwin(z_v, t, 0, 0), scalar1=wv_tap(0),
                scalar2=bias_v, op0=OP.mult, op1=OP.add)
            for tap in range(1, 9):
                dy, dx = divmod(tap, 3)
                nacc = sm.tile([128, TS], F16, tag="accv", name="accv")
                nc.vector.scalar_tensor_tensor(
                    out=nacc[:], in0=zwin(z_v, t, dy, dx),
                    scalar=wv_tap(tap), in1=acc[:],
                    op0=OP.mult, op1=OP.add)
                acc = nacc
        return acc

    # ---------------- pipelined loops ----------------
    def loop_a(b):
        """dwconv + gating chain, 2-tile skew."""
        z_q, z_k, z_v = zbufs[b % 2]
        cat_hi = cat_hi2[b % 2]
        qk_t = {}
        hs = {}
        for i in range(NT + 2):
            if i < NT:
                t = i
                pdq = dw_mm8(z_q, 0, t, "pdq")
                q_t = sm.tile([128, TS], F16, tag="q_t", name="q_t")
                nc.scalar.activation(out=q_t[:], in_=pdq[:],
                                     func=AF.Identity, bias=bias_q)
                pdk = dw_mm8(z_k, 1, t, "pdk")
                qk = sm.tile([128, TS], F16, tag="qk_t", name="qk_t")
                with nc.allow_low_precision(reason="qk product fits fp16"):
                    nc.vector.scalar_tensor_tensor(
                        out=qk[:], in0=pdk[:], scalar=bias_k, in1=q_t[:],
                        op0=OP.add, op1=OP.mult)
                qk_t[t] = qk
            if 1 <= i <= NT:
                t = i - 1
                pa1 = ps.tile([128, TS], F32, tag="py", name="pa1")
                nc.tensor.matmul(pa1[:], wact1[:], qk_t[t][:],
                                 start=True, stop=True)
                t_a = sm.tile([128, TS], F16, tag="t_a", name="t_a")
                nc.scalar.activation(out=t_a[:], in_=pa1[:],
                                     func=AF.Identity, bias=bact1)
                u_t = sm.tile([128, TS], F16, tag="u_t", name="u_t")
                nc.scalar.activation(out=u_t[:], in_=t_a[:],
                                     func=AF.Relu, bias=const3)
                h_t = sm.tile([128, TS], F16, tag="hs_t", name="hs_t")
                with nc.allow_low_precision(reason="hardswish fits fp16"):
                    nc.vector.scalar_tensor_tensor(
                        out=h_t[:], in0=u_t[:], scalar=6.0, in1=t_a[:],
                        op0=OP.min, op1=OP.mult)
                hs[t] = h_t
            if 2 <= i:
                t = i - 2
                sl = slice(t * TS, (t + 1) * TS)
                pa2 = ps.tile([128, TS], F32, tag="py", name="pa2")
                nc.tensor.matmul(pa2[:], wact2[:], hs[t][:],
                                 start=True, stop=True)
                g_t = sm.tile([128, TS], F16, tag="g_t", name="g_t")
                nc.scalar.activation(out=g_t[:], in_=pa2[:], func=AF.Tanh,
                                     bias=bact2)
                if t in OFFV:
                    acc = dwv_vector(z_v, t)
                    with nc.allow_low_precision(reason="gated out fp16"):
                        nc.vector.scalar_tensor_tensor(
                            out=cat_hi[:, sl], in0=acc[:], scalar=1.0,
                            in1=g_t[:], op0=OP.mult, op1=OP.mult)
                else:
                    pdv = dw_mm(z_v, 2, t, "pdv")
                    with nc.allow_low_precision(reason="gated out fp16"):
                        nc.vector.scalar_tensor_tensor(
                            out=cat_hi[:, sl], in0=pdv[:], scalar=bias_v,
                            in1=g_t[:], op0=OP.add, op1=OP.mult)

    def scores_stage(par, t, gq_sb):
        sl = slice(t * TS, (t + 1) * TS)
        for p in range(2):
            pat = ps.tile([128, TS], F32, tag="px", name="pat")
            nc.tensor.matmul(pat[:], gk2[par][p][:], gq_sb[:, sl],
                             start=True, stop=True)
            nc.scalar.activation(out=exp_sb[p][:, sl], in_=pat[:],
                                 func=AF.Exp, scale=float(SCALER))

    def den_stage(t):
        sl = slice(t * TS, (t + 1) * TS)
        pden = ps.tile([128, TS], F32, tag="px", name="pden")
        for p in range(2):
            nc.tensor.matmul(pden[:], denmask[p][:], exp_sb[p][:, sl],
                             start=(p == 0), stop=(p == 1))
        nc.vector.reciprocal_approx_fast(out=rec_rep[:, sl], in_=pden[:])

    def av_stage(t, par, cat_lo):
        sl = slice(t * TS, (t + 1) * TS)
        av0, av1 = av_lhs[par]
        pav = ps.tile([128, TS], F32, tag="px", name="pav")
        nc.tensor.matmul(pav[:], av0[:], exp_sb[0][:, sl],
                         start=True, stop=False)
        nc.tensor.matmul(pav[:], av1[:], exp_sb[1][:, sl],
                         start=False, stop=True)
        with nc.allow_low_precision(reason="attn out fits fp16"):
            nc.vector.scalar_tensor_tensor(
                out=cat_lo[:, sl], in0=pav[:], scalar=1.0,
                in1=rec_rep[:, sl], op0=OP.mult, op1=OP.mult)

    def proj_stage(b, t, cat_hi, cat_lo):
        sl = slice(t * TS, (t + 1) * TS)
        for m in range(2):
            pp = ps.tile([128, TS], F32, tag="py", name="pp")
            nc.tensor.matmul(pp[:], wproj[0][:, m * 128:(m + 1) * 128],
                             cat_hi[:, sl], start=True, stop=False)
            nc.tensor.matmul(pp[:], wproj[1][:, m * 128:(m + 1) * 128],
                             cat_lo[:, sl], start=False, stop=True)
            o_t = sm.tile([128, TS], F16, tag=f"o_t{m}", name=f"o_t{m}")
            nc.scalar.copy(out=o_t[:], in_=pp[:])
            nc.sync.dma_start(out=d["out"][b, m, :, sl], in_=o_t)

    def pool_closures(x_sb):
        """4 GPSIMD-free pooling ops as closures, sprinkled across stages."""
        ops = []
        pooled = []
        for cc in range(2):
            pr1 = sm.tile([128, H * HP], F32, tag="pr1", name="pr1")
            po = tiny.tile([128, POOL_N], F16, tag="po", name="po")
            pooled.append(po)

            def st1(cc=cc, pr1=pr1):
                nc.vector.tensor_reduce(
                    out=pr1.rearrange("p (y g) -> p y g", g=HP),
                    in_=x_sb[cc].rearrange("p (y g x) -> p y g x",
                                           y=H, g=HP),
                    axis=mybir.AxisListType.X, op=OP.add)

            def st2(pr1=pr1, po=po):
                with nc.allow_low_precision(reason="pool sums fit fp16"):
                    nc.vector.tensor_reduce(
                        out=po.rearrange("p (a b) -> p a b", a=HP),
                        in_=pr1.rearrange("p (hp dy wp) -> p hp wp dy",
                                          hp=HP, dy=WIN),
                        axis=mybir.AxisListType.X, op=OP.add)
            ops.append(st1)
            ops.append(st2)
        return ops, pooled

    def unified(b, x_next):
        """attention + projection for image b fully interleaved with the
        qkv/gq front-end and dw/gating chain of image b+1."""
        par = b % 2
        npar = 1 - par
        cat_hi = cat_hi2[par]
        cat_lo = cat_lo2[par]
        gq_sb = gq_sb2[par]
        nz_q, nz_k, nz_v = zbufs[npar]
        ncat_hi = cat_hi2[npar]
        qk_t = {}
        hs = {}
        have = x_next is not None
        if have:
            pops, pooled = pool_closures(x_next)
        for i in range(NT + 5):
            if have and i < NT:
                qkv_tile(b + 1, i, x_next)
                gq_tile(b + 1, i, x_next)
            if i < NT:
                scores_stage(par, i, gq_sb)
            if have and 2 <= i < NT + 2:
                t = i - 2
                pdq = dw_mm8(nz_q, 0, t, "pdq")
                q_t = sm.tile([128, TS], F16, tag="q_t", name="q_t")
                nc.scalar.activation(out=q_t[:], in_=pdq[:],
                                     func=AF.Identity, bias=bias_q)
                pdk = dw_mm8(nz_k, 1, t, "pdk")
                qk = sm.tile([128, TS], F16, tag="qk_t", name="qk_t")
                with nc.allow_low_precision(reason="qk product fits fp16"):
                    nc.vector.scalar_tensor_tensor(
                        out=qk[:], in0=pdk[:], scalar=bias_k, in1=q_t[:],
                        op0=OP.add, op1=OP.mult)
                qk_t[t] = qk
            if 1 <= i <= NT:
                den_stage(i - 1)
            if have and 3 <= i < NT + 3:
                t = i - 3
                pa1 = ps.tile([128, TS], F32, tag="py", name="pa1")
                nc.tensor.matmul(pa1[:], wact1[:], qk_t[t][:],
                                 start=True, stop=True)
                t_a = sm.tile([128, TS], F16, tag="t_a", name="t_a")
                nc.scalar.activation(out=t_a[:], in_=pa1[:],
                                     func=AF.Identity, bias=bact1)
                u_t = sm.tile([128, TS], F16, tag="u_t", name="u_t")
                nc.scalar.activation(out=u_t[:], in_=t_a[:],
                                     func=AF.Relu, bias=const3)
                h_t = sm.tile([128, TS], F16, tag="hs_t", name="hs_t")
                with nc.allow_low_precision(reason="hardswish fits fp16"):
                    nc.vector.scalar_tensor_tensor(
                        out=h_t[:], in0=u_t[:], scalar=6.0, in1=t_a[:],
                        op0=OP.min, op1=OP.mult)
                hs[t] = h_t
            if 2 <= i <= NT + 1:
                av_stage(i - 2, par, cat_lo)
            if have and 4 <= i < NT + 4:
                t = i - 4
                sl = slice(t * TS, (t + 1) * TS)
                pa2 = ps.tile([128, TS], F32, tag="py", name="pa2")
                nc.tensor.matmul(pa2[:], wact2[:], hs[t][:],
                                 start=True, stop=True)
                g_t = sm.tile([128, TS], F16, tag="g_t", name="g_t")
                nc.scalar.activation(out=g_t[:], in_=pa2[:], func=AF.Tanh,
                                     bias=bact2)
                if t in OFFV:
                    acc = dwv_vector(nz_v, t)
                    with nc.allow_low_precision(reason="gated out fp16"):
                        nc.vector.scalar_tensor_tensor(
                            out=ncat_hi[:, sl], in0=acc[:], scalar=1.0,
                            in1=g_t[:], op0=OP.mult, op1=OP.mult)
                else:
                    pdv = dw_mm(nz_v, 2, t, "pdv")
                    with nc.allow_low_precision(reason="gated out fp16"):
                        nc.vector.scalar_tensor_tensor(
                            out=ncat_hi[:, sl], in0=pdv[:], scalar=bias_v,
                            in1=g_t[:], op0=OP.add, op1=OP.mult)
            if 3 <= i <= NT + 2:
                proj_stage(b, i - 3, cat_hi, cat_lo)
            if have and 2 <= i <= 5:
                pops[i - 2]()
        if have:
            pool_finish(b + 1, pooled)

    # ---------------- program ----------------
    x_cur = load_x(0)
    pooled = pool_reduce(x_cur)
    for t in range(NT):
        qkv_tile(0, t, x_cur)
        gq_tile(0, t, x_cur)
    pool_finish(0, pooled)
    loop_a(0)

    for b in range(n_img):
        x_next = load_x(b + 1) if b + 1 < n_img else None
        unified(b, x_next)


def _build(n_img=B):
    nc = bacc.Bacc("TRN2", target_bir_lowering=False, debug=False,
                   num_devices=N_CORES)
    dt = nc.dram_tensor
    d = {
        "x": dt("x", [B, 2, 128, HW], F16, kind="ExternalInput").ap(),
        "x8": dt("x8", [B, 128, 2 * HW], F8, kind="ExternalInput").ap(),
        "wf16": dt("wf16", [128, W16_COLS], F16, kind="ExternalInput").ap(),
        "wf32": dt("wf32", [128, W32_COLS], F32, kind="ExternalInput").ap(),
        "wf8": dt("wf8", [128, W8_COLS], F8, kind="ExternalInput").ap(),
        "out": dt("out", [B, 2, 128, HW], F16, kind="ExternalOutput").ap(),
    }
    with tile.TileContext(nc) as tc, ExitStack() as ctx:
        _body(ctx, tc, d, n_img=n_img)
    nc.compile()
    return nc


_NC = None


def _prep_weights(qkv_w, dw_w, dw_b, act1_w, act1_b, act2_w, act2_b,
                  gq_w, gkv_w, proj_w):
    f32 = np.float32
    f16 = np.float16
    sc = np.float32(HEAD_DIM ** -0.5)

    wqkv = qkv_w.T.reshape(2, 128, 384).astype(f16)
    taps = dw_w.reshape(384, 9)            # [c, tap]
    idx = np.arange(128)
    # f16 diag blocks for the v branch only
    dwd = np.zeros((9, 128, 128), dtype=f16)
    for tp in range(9):
        dwd[tp, idx, idx] = taps[256:384, tp]
    dwdiag = dwd.transpose(1, 0, 2).reshape(128, 9 * 128)
    # fp8 DoubleRow pair blocks for q and k branches
    f8 = np.dtype(np.float32)  # placeholder; real cast below
    import ml_dtypes
    e4 = ml_dtypes.float8_e4m3
    wf8 = np.zeros((128, W8_COLS), dtype=e4)
    for br in range(2):
        tb = taps[128 * br:128 * (br + 1)]
        for pr, (ta_, tb_) in enumerate(DW_PAIRS):
            blk = np.zeros((128, 2, 128), np.float32)
            blk[idx, 0, idx] = tb[:, ta_]
            blk[idx, 1, idx] = tb[:, tb_]
            wf8[:, W8_DW + br * W8_BRANCH + pr * 256:
                W8_DW + br * W8_BRANCH + (pr + 1) * 256] = (
                blk.reshape(128, 256).astype(e4))
        t8 = np.zeros((128, 128), np.float32)
        t8[idx, idx] = tb[:, 8]
        wf8[:, W8_DW + br * W8_BRANCH + 1024:
            W8_DW + br * W8_BRANCH + 1152] = t8.astype(e4)
    for bi, wsrc in enumerate((qkv_w[0:128], qkv_w[128:256], gq_w)):
        blk = (wsrc.T.reshape(2, 128, 128).transpose(1, 0, 2)
               .reshape(128, 256))
        wf8[:, W8_QKV + bi * 256:W8_QKV + (bi + 1) * 256] = blk.astype(e4)
    wact1 = (act1_w * sc).T.astype(f16)
    wact2 = (act2_w / 6.0).T.astype(f16)
    wgq = gq_w.T.reshape(2, 128, 128).astype(f16)
    wgkv = (gkv_w / 49.0).T.reshape(2, 128, 256).astype(f16)
    wproj = proj_w.T.reshape(2, 128, 256).astype(f16)
    dm = np.zeros((2, 128, 128), dtype=f16)
    for p in range(2):
        for hl in range(2):
            head = 2 * p + hl
            dm[p, 64 * hl:64 * hl + 64, 32 * head:32 * head + 32] = 1.0

    blocks = {"wqkv0": wqkv[0], "wqkv1": wqkv[1], "dwdiag": dwdiag,
              "wact1": wact1, "wact2": wact2, "wgq0": wgq[0],
              "wgq1": wgq[1], "wgkv0": wgkv[0], "wgkv1": wgkv[1],
              "wproj0": wproj[0], "wproj1": wproj[1],
              "denmask0": dm[0], "denmask1": dm[1]}
    wf16 = np.zeros((128, W16_COLS), dtype=f16)
    for nm, (a, b_) in WCOL.items():
        wf16[:, a:b_] = blocks[nm]

    wf32 = np.zeros((128, W32_COLS), dtype=f32)
    wf32[:, 0:3] = dw_b.reshape(3, 128).T
    wf32[:, 3] = act1_b.astype(f32)
    wf32[:, 4] = act2_b.astype(f32)
    wf32[:, 5:14] = taps[256:384].astype(f32)   # dw-v taps for DVE path
    wf32[:, 14] = 3.0

    return {"wf16": np.ascontiguousarray(wf16),
            "wf32": np.ascontiguousarray(wf32),
            "wf8": np.ascontiguousarray(wf8)}


def _make_in_maps(inputs):
    w = _prep_weights(
        inputs["qkv_w"], inputs["dw_w"], inputs["dw_b"],
        inputs["act1_w"], inputs["act1_b"], inputs["act2_w"],
        inputs["act2_b"], inputs["gq_w"], inputs["gkv_w"],
        inputs["proj_w"])
    x = inputs["x"]
    in_maps = []
    for core in range(N_CORES):
        m = dict(w)
        xc = x[core * B:(core + 1) * B]
        m["x"] = np.ascontiguousarray(
            xc.reshape(B, 2, 128, HW).astype(np.float16))
        import ml_dtypes
        m["x8"] = np.ascontiguousarray(
            xc.reshape(B, 2, 128, HW).transpose(0, 2, 1, 3)
            .reshape(B, 128, 2 * HW).astype(ml_dtypes.float8_e4m3))
        in_maps.append(m)
    return in_maps


def kernel(**inputs):
    global _NC
    if _NC is None:
        _NC = _build()
    in_maps = _make_in_maps(inputs)
    res = run_bass_kernel_spmd(_NC, in_maps, core_ids=list(range(N_CORES)))
    out = np.concatenate([r["out"] for r in res.results], axis=0)
    return out.reshape(B_FULL, C, H, W).astype(np.float32)


# revision 40
# speedup vs baseline: 1.0134x; 1.0037x over previous
"""CloAttention Trainium2 Bass kernel.

Full inputs -> data-parallel over batch across 8 NeuronCores (4 images each)
-> full output.  All matmuls run on the PE in fp16 (1 cycle/row); the 3x3
depthwise conv runs as 9 diagonal-matmul accumulations into PSUM.

Schedule: software-pipelined per image.  Loop A runs the depthwise/gating
chain with a 2-tile skew so the PE never waits on the scalar/vector chain;
loop B runs attention + projection for image b interleaved with the qkv/gq
front-end of image b+1.  Pooling runs on the otherwise-idle GPSIMD engine;
a couple of dw-v tiles per image run as shifted multiply-adds on DVE to
shave PE work.  Weights arrive in two consolidated DMAs and dummy matmuls
warm the PE HAM clock-gate during the initial DMA wait.
"""

import numpy as np
from contextlib import ExitStack

import concourse.bacc as bacc
import concourse.bass as bass
import concourse.tile as tile
from concourse import mybir
from concourse.bass_utils import run_bass_kernel_spmd

F32 = mybir.dt.float32
F16 = mybir.dt.float16
F8 = mybir.dt.float8e4
AF = mybir.ActivationFunctionType
OP = mybir.AluOpType
DR = mybir.MatmulPerfMode.DoubleRow

# dw tap pairing for fp8 DoubleRow matmuls: 4 pairs with a constant
# address delta between the two shifted windows, plus tap 8 standalone
DW_PAIRS = ((0, 1), (3, 4), (6, 7), (2, 5))
W8_BRANCH = 4 * 256 + 128      # cols per branch in the fp8 weight block
W8_QKV = 0                     # qkv DR blocks first (DR LDW needs low offs)
W8_DW = 3 * 256                # dw branches after
W8_COLS = W8_DW + 2 * W8_BRANCH

N_CORES = 8
B_FULL = 32
B = B_FULL // N_CORES          # images per core
C = 256
H = W = 56
HW = H * W                     # 3136
PW = H + 2                     # 58 padded
NT = 7                         # pixel tiles per image
TS = HW // NT                  # 448 = 8 rows of 56
RPT = H // NT                  # 8 rows per tile
HEAD_DIM = 32
SCALER = HEAD_DIM ** -0.5
WIN = 7
HP = H // WIN                  # 8
POOL_N = HP * HP               # 64

OFFV = (1, 4)                  # tiles whose dw-v runs on DVE, not PE

# f16 weight block column offsets
WCOL = {}
_off = 0
for _nm, _w in (("wqkv0", 384), ("wqkv1", 384), ("dwdiag", 1152),
                ("wact1", 128), ("wact2", 128), ("wgq0", 128),
                ("wgq1", 128), ("wgkv0", 256), ("wgkv1", 256),
                ("wproj0", 256), ("wproj1", 256), ("denmask0", 128),
                ("denmask1", 128)):
    WCOL[_nm] = (_off, _off + _w)
    _off += _w
W16_COLS = _off                # 6016
W32_COLS = 15                  # dwb q,k,v | bact1 | bact2 | dwv taps 0..8 | 3.0


def _body(ctx, tc, d, n_img=B):
    nc = tc.nc

    # ---------------- persistent weights (2 consolidated DMAs) ----------
    wpool = ctx.enter_context(tc.tile_pool(name="wpool", bufs=1))

    warm_src = wpool.tile([128, 64], F16, tag="warm_src", name="warm_src")
    nc.vector.memset(warm_src, 0.0)

    wf16 = wpool.tile([128, W16_COLS], F16, tag="wf16", name="wf16")
    nc.sync.dma_start(out=wf16, in_=d["wf16"])
    wf32 = wpool.tile([128, W32_COLS], F32, tag="wf32", name="wf32")
    nc.sync.dma_start(out=wf32, in_=d["wf32"])
    wf8 = wpool.tile([128, W8_COLS], F8, tag="wf8", name="wf8")
    nc.sync.dma_start(out=wf8, in_=d["wf8"])

    def wv(name):
        a, b_ = WCOL[name]
        return wf16[:, a:b_]

    wqkv = [wv("wqkv0"), wv("wqkv1")]
    dwdiag = wv("dwdiag")
    wact1 = wv("wact1")
    wact2 = wv("wact2")
    wgq = [wv("wgq0"), wv("wgq1")]
    wgkv = [wv("wgkv0"), wv("wgkv1")]
    wproj = [wv("wproj0"), wv("wproj1")]
    denmask = [wv("denmask0"), wv("denmask1")]
    bias_q = wf32[:, 0:1]
    bias_k = wf32[:, 1:2]
    bias_v = wf32[:, 2:3]
    bact1 = wf32[:, 3:4]
    bact2 = wf32[:, 4:5]
    const3 = wf32[:, 14:15]

    def w8dr(idx):
        """fp8 DoubleRow lhsT [128, 2, 128] for qkv q(0) / k(1) / gq(2)."""
        off = W8_QKV + idx * 256
        return wf8[:, off:off + 256].rearrange("p (i m) -> p i m", i=2)

    def wv_tap(tap):
        return wf32[:, 5 + tap:6 + tap]

    def dw_lhsT(cc, tap):
        return dwdiag[:, tap * 128:(tap + 1) * 128]    # v branch only

    # padded z buffers, x2 for image parity (borders stay zero; interiors
    # rewritten per image).  q/k are fp8 (read only by the DoubleRow dw
    # matmuls); v stays fp16.
    zbufs = []
    for par in range(2):
        zs = [wpool.tile([128, PW * PW], F8 if j < 2 else F16,
                         tag=f"z{j}_{par}", name=f"z{j}_{par}")
              for j in range(3)]
        for z in zs:
            zg = z.rearrange("p (r c) -> p r c", c=PW)
            nc.vector.memset(zg[:, 0, :], 0.0)          # top border row
            nc.vector.memset(zg[:, PW - 1, :], 0.0)     # bottom border row
            nc.vector.memset(zg[:, :, 0], 0.0)          # left border col
            nc.vector.memset(zg[:, :, PW - 1], 0.0)     # right border col
        zbufs.append(zs)

    # block-diagonal gk (2 heads per matmul at K=128) and zero-padded AV
    # lhsT blocks, x2 parity; zero regions never rewritten -> memset once
    gk2 = []
    av_lhs = []
    for par in range(2):
        g = [wpool.tile([128, 128], F16, tag=f"gk2_{p}_{par}",
                        name=f"gk2_{p}_{par}") for p in range(2)]
        a = [wpool.tile([128, 128], F16, tag=f"av_{p}_{par}",
                        name=f"av_{p}_{par}") for p in range(2)]
        for tbuf in (*g, *a):
            nc.vector.memset(tbuf, 0.0)
        gk2.append(g)
        av_lhs.append(a)

    # ---------------- pools ----------------
    ps = ctx.enter_context(tc.tile_pool(name="ps", bufs=4, space="PSUM"))
    xpool = ctx.enter_context(tc.tile_pool(name="xpool", bufs=2))
    big = ctx.enter_context(tc.tile_pool(name="big", bufs=1))
    sm = ctx.enter_context(tc.tile_pool(name="sm", bufs=4))
    tiny = ctx.enter_context(tc.tile_pool(name="tiny", bufs=2))

    gq_sb2 = [big.tile([128, HW], F16, tag=f"gq_sb{i}", name=f"gq_sb{i}")
              for i in range(2)]
    exp_sb = [big.tile([128, HW], F16, tag=f"exp{p}", name=f"exp{p}")
              for p in range(2)]
    rec_rep = big.tile([128, HW], F32, tag="rec_rep")
    cat_hi2 = [big.tile([128, HW], F16, tag=f"cat_hi{i}", name=f"cat_hi{i}")
               for i in range(2)]
    cat_lo2 = [big.tile([128, HW], F16, tag=f"cat_lo{i}", name=f"cat_lo{i}")
               for i in range(2)]

    zgrid = {id(z): z.rearrange("p (r c) -> p r c", c=PW)
             for zs in zbufs for z in zs}

    def zwin(z, t, dy, dx):
        r0 = RPT * t + dy
        return zgrid[id(z)][:, r0:r0 + RPT, dx:dx + W]

    def zint(z, t):
        r0 = RPT * t + 1
        return zgrid[id(z)][:, r0:r0 + RPT, 1:1 + W]

    # PE warmup: dummy matmuls keep the HAM clock-gate busy while the
    # weight/x DMAs land, so real matmuls start at 2.4 GHz
    for wi in range(72):
        pw = ps.tile([64, 64], F32, tag="py", name="pwarm")
        nc.tensor.matmul(pw[:], warm_src[:], warm_src[:],
                         start=True, stop=True)

    # ---------------- stage helpers ----------------
    def load_x(b):
        x_sb = [xpool.tile([128, HW], F16, tag=f"x{cc}", name=f"x{cc}")
                for cc in range(2)]
        for cc in range(2):
            nc.sync.dma_start(out=x_sb[cc], in_=d["x"][b, cc])
        x8 = xpool.tile([128, 2 * HW], F8, tag="x8", name="x8")
        nc.sync.dma_start(out=x8, in_=d["x8"][b])
        # [p, 2, rows, 56]: the interleave-of-2 must be the third free dim
        # from the innermost (DoubleRow ISA requires n_elem[2]==2)
        x_sb.append(x8.rearrange("p (i r c) -> p i r c", i=2, c=W))
        return x_sb

    def qkv_tile(b, t, x_sb):
        z_q, z_k, z_v = zbufs[b % 2]
        rhs8 = x_sb[2][:, :, t * RPT:(t + 1) * RPT, :]
        for j, (z, eng) in enumerate(
                ((z_q, "act"), (z_k, "act"), (z_v, "dve"))):
            pq = ps.tile([128, TS], F32, tag="py", name="pq")
            if j < 2:
                nc.tensor.matmul(pq[:], w8dr(j), rhs8,
                                 start=True, stop=True, perf_mode=DR)
            else:
                for cc in range(2):
                    nc.tensor.matmul(
                        pq[:], wqkv[cc][:, 256:384],
                        x_sb[cc][:, t * TS:(t + 1) * TS],
                        start=(cc == 0), stop=(cc == 1))
            if eng == "act":
                nc.scalar.copy(out=zint(z, t), in_=pq[:])
            else:
                nc.vector.tensor_copy(out=zint(z, t), in_=pq[:])

    def gq_tile(b, t, x_sb):
        pg = ps.tile([128, TS], F32, tag="py", name="pg")
        for cc in range(2):
            nc.tensor.matmul(pg[:], wgq[cc][:],
                             x_sb[cc][:, t * TS:(t + 1) * TS],
                             start=(cc == 0), stop=(cc == 1))
        nc.vector.tensor_copy(out=gq_sb2[b % 2][:, t * TS:(t + 1) * TS],
                              in_=pg[:])

    def pool_reduce(x_sb):
        """7x7 window sums; issued early so the results have slack."""
        pooled = []
        for cc in range(2):
            pr1 = sm.tile([128, H * HP], F32, tag="pr1", name="pr1")
            nc.vector.tensor_reduce(
                out=pr1.rearrange("p (y g) -> p y g", g=HP),
                in_=x_sb[cc].rearrange("p (y g x) -> p y g x", y=H, g=HP),
                axis=mybir.AxisListType.X, op=OP.add)
            po = tiny.tile([128, POOL_N], F16, tag="po", name="po")
            with nc.allow_low_precision(reason="pool sums fit fp16"):
                nc.vector.tensor_reduce(
                    out=po.rearrange("p (a b) -> p a b", a=HP),
                    in_=pr1.rearrange("p (hp dy wp) -> p hp wp dy",
                                      hp=HP, dy=WIN),
                    axis=mybir.AxisListType.X, op=OP.add)
            pooled.append(po)
        return pooled

    def pool_finish(b, pooled):
        """global-kv matmuls + lhsT packing for image b's attention."""
        par = b % 2
        pgk = ps.tile([128, POOL_N], F32, tag="py", name="pgk")
        for cc in range(2):
            nc.tensor.matmul(pgk[:], wgkv[cc][:, 0:128], pooled[cc][:],
                             start=(cc == 0), stop=(cc == 1))
        for p in range(2):
            for hl in range(2):
                h = 2 * p + hl
                nc.scalar.copy(
                    out=gk2[par][p][32 * h:32 * h + 32,
                                    64 * hl:64 * hl + 64],
                    in_=pgk[32 * h:32 * h + 32, :])
        pgv = ps.tile([POOL_N, 128], F32, tag="py", name="pgv")
        for cc in range(2):
            nc.tensor.matmul(pgv[:], pooled[cc][:], wgkv[cc][:, 128:256],
                             start=(cc == 0), stop=(cc == 1))
        gvT = tiny.tile([POOL_N, 128], F16, tag="gvT", name="gvT")
        nc.scalar.copy(out=gvT[:], in_=pgv[:])
        av0, av1 = av_lhs[par]
        nc.vector.tensor_copy(out=av0[0:64, 0:32], in_=gvT[:, 0:32])
        nc.sync.dma_start(out=av0[64:128, 32:64], in_=gvT[:, 32:64])
        nc.vector.tensor_copy(out=av1[0:64, 64:96], in_=gvT[:, 64:96])
        nc.sync.dma_start(out=av1[64:128, 96:128], in_=gvT[:, 96:128])

    def dw_mm(z, cc, t, psname):
        p = ps.tile([128, TS], F32, tag="px", name=psname)
        for tap in range(9):
            dy, dx = divmod(tap, 3)
            nc.tensor.matmul(p[:], dw_lhsT(cc, tap), zwin(z, t, dy, dx),
                             start=(tap == 0), stop=(tap == 8))
        return p

    def dw_mm8(z8, br, t, psname):
        """dw conv via 4 fp8 DoubleRow pair-matmuls + 1 plain fp8 matmul."""
        p = ps.tile([128, TS], F32, tag="px", name=psname)
        zg = zgrid[id(z8)]
        for pr, (tapA, tapB) in enumerate(DW_PAIRS):
            dyA, dxA = divmod(tapA, 3)
            dyB, dxB = divmod(tapB, 3)
            delta = (dyB - dyA) * PW + (dxB - dxA)
            w = zg[:, RPT * t + dyA:RPT * t + dyA + RPT, dxA:dxA + W]
            pa = list(w.ap)
            rhs = bass.AP(w.tensor, w.offset,
                          [pa[0], [delta, 2], pa[1], pa[2]])
            lhsT = wf8[:, W8_DW + br * W8_BRANCH + pr * 256:
                       W8_DW + br * W8_BRANCH + (pr + 1) * 256]
            nc.tensor.matmul(p[:], lhsT.rearrange("p (i m) -> p i m", i=2),
                             rhs, start=(pr == 0), stop=False,
                             perf_mode=mybir.MatmulPerfMode.DoubleRow)
        nc.tensor.matmul(p[:],
                         wf8[:, W8_DW + br * W8_BRANCH + 1024:
                             W8_DW + br * W8_BRANCH + 1152],
                         zwin(z8, t, 2, 2), start=False, stop=True)
        return p

    def dwv_vector(z_v, t):
        """dw-v for one tile as 9 shifted multiply-adds on DVE; returns
        the accumulated (dwv + bias_v) tile in fp16."""
        acc = sm.tile([128, TS], F16, tag="accv", name="accv")
        with nc.allow_low_precision(reason="dwv fits fp16"):
            nc.vector.tensor_scalar(
                out=acc[:], in0=zwin(z_v, t, 0, 0), scalar1=wv_tap(0),
                scalar2=bias_v, op0=OP.mult, op1=OP.add)
            for tap in range(1, 9):
                dy, dx = divmod(tap, 3)
                nacc = sm.tile([128, TS], F16, tag="accv", name="accv")
                nc.vector.scalar_tensor_tensor(
                    out=nacc[:], in0=zwin(z_v, t, dy, dx),
                    scalar=wv_tap(tap), in1=acc[:],
                    op0=OP.mult, op1=OP.add)
                acc = nacc
        return acc

    # ---------------- pipelined loops ----------------
    def loop_a(b):
        """dwconv + gating chain, 2-tile skew."""
        z_q, z_k, z_v = zbufs[b % 2]
        cat_hi = cat_hi2[b % 2]
        qk_t = {}
        hs = {}
        for i in range(NT + 2):
            if i < NT:
                t = i
                pdq = dw_mm8(z_q, 0, t, "pdq")
                q_t = sm.tile([128, TS], F16, tag="q_t", name="q_t")
                nc.scalar.activation(out=q_t[:], in_=pdq[:],
                                     func=AF.Identity, bias=bias_q)
                pdk = dw_mm8(z_k, 1, t, "pdk")
                qk = sm.tile([128, TS], F16, tag="qk_t", name="qk_t")
                with nc.allow_low_precision(reason="qk product fits fp16"):
                    nc.vector.scalar_tensor_tensor(
                        out=qk[:], in0=pdk[:], scalar=bias_k, in1=q_t[:],
                        op0=OP.add, op1=OP.mult)
                qk_t[t] = qk
            if 1 <= i <= NT:
                t = i - 1
                pa1 = ps.tile([128, TS], F32, tag="py", name="pa1")
                nc.tensor.matmul(pa1[:], wact1[:], qk_t[t][:],
                                 start=True, stop=True)
                t_a = sm.tile([128, TS], F16, tag="t_a", name="t_a")
                nc.scalar.activation(out=t_a[:], in_=pa1[:],
                                     func=AF.Identity, bias=bact1)
                u_t = sm.tile([128, TS], F16, tag="u_t", name="u_t")
                nc.scalar.activation(out=u_t[:], in_=t_a[:],
                                     func=AF.Relu, bias=const3)
                h_t = sm.tile([128, TS], F16, tag="hs_t", name="hs_t")
                with nc.allow_low_precision(reason="hardswish fits fp16"):
                    nc.vector.scalar_tensor_tensor(
                        out=h_t[:], in0=u_t[:], scalar=6.0, in1=t_a[:],
                        op0=OP.min, op1=OP.mult)
                hs[t] = h_t
            if 2 <= i:
                t = i - 2
                sl = slice(t * TS, (t + 1) * TS)
                pa2 = ps.tile([128, TS], F32, tag="py", name="pa2")
                nc.tensor.matmul(pa2[:], wact2[:], hs[t][:],
                                 start=True, stop=True)
                g_t = sm.tile([128, TS], F16, tag="g_t", name="g_t")
                nc.scalar.activation(out=g_t[:], in_=pa2[:], func=AF.Tanh,
                                     bias=bact2)
                if t in OFFV:
                    acc = dwv_vector(z_v, t)
                    with nc.allow_low_precision(reason="gated out fp16"):
                        nc.vector.scalar_tensor_tensor(
                            out=cat_hi[:, sl], in0=acc[:], scalar=1.0,
                            in1=g_t[:], op0=OP.mult, op1=OP.mult)
                else:
                    pdv = dw_mm(z_v, 2, t, "pdv")
                    with nc.allow_low_precision(reason="gated out fp16"):
                        nc.vector.scalar_tensor_tensor(
                            out=cat_hi[:, sl], in0=pdv[:], scalar=bias_v,
                            in1=g_t[:], op0=OP.add, op1=OP.mult)

    def scores_stage(par, t, gq_sb):
        sl = slice(t * TS, (t + 1) * TS)
        for p in range(2):
            pat = ps.tile([128, TS], F32, tag="px", name="pat")
            nc.tensor.matmul(pat[:], gk2[par][p][:], gq_sb[:, sl],
                             start=True, stop=True)
            nc.scalar.activation(out=exp_sb[p][:, sl], in_=pat[:],
                                 func=AF.Exp, scale=float(SCALER))

    def den_stage(t):
        sl = slice(t * TS, (t + 1) * TS)
        pden = ps.tile([128, TS], F32, tag="px", name="pden")
        for p in range(2):
            nc.tensor.matmul(pden[:], denmask[p][:], exp_sb[p][:, sl],
                             start=(p == 0), stop=(p == 1))
        nc.vector.reciprocal_approx_fast(out=rec_rep[:, sl], in_=pden[:])

    def av_stage(t, par, cat_lo):
        sl = slice(t * TS, (t + 1) * TS)
        av0, av1 = av_lhs[par]
        pav = ps.tile([128, TS], F32, tag="px", name="pav")
        nc.tensor.matmul(pav[:], av0[:], exp_sb[0][:, sl],
                         start=True, stop=False)
        nc.tensor.matmul(pav[:], av1[:], exp_sb[1][:, sl],
                         start=False, stop=True)
        with nc.allow_low_precision(reason="attn out fits fp16"):
            nc.vector.scalar_tensor_tensor(
                out=cat_lo[:, sl], in0=pav[:], scalar=1.0,
                in1=rec_rep[:, sl], op0=OP.mult, op1=OP.mult)

    def proj_stage(b, t, cat_hi, cat_lo):
        sl = slice(t * TS, (t + 1) * TS)
        for m in range(2):
            pp = ps.tile([128, TS], F32, tag="py", name="pp")
            nc.tensor.matmul(pp[:], wproj[0][:, m * 128:(m + 1) * 128],
                             cat_hi[:, sl], start=True, stop=False)
            nc.tensor.matmul(pp[:], wproj[1][:, m * 128:(m + 1) * 128],
                             cat_lo[:, sl], start=False, stop=True)
            o_t = sm.tile([128, TS], F16, tag=f"o_t{m}", name=f"o_t{m}")
            if m == 0:
                nc.scalar.copy(out=o_t[:], in_=pp[:])
            else:
                nc.vector.tensor_copy(out=o_t[:], in_=pp[:])
            nc.sync.dma_start(out=d["out"][b, m, :, sl], in_=o_t)

    def pool_closures(x_sb):
        """4 GPSIMD-free pooling ops as closures, sprinkled across stages."""
        ops = []
        pooled = []
        for cc in range(2):
            pr1 = sm.tile([128, H * HP], F32, tag="pr1", name="pr1")
            po = tiny.tile([128, POOL_N], F16, tag="po", name="po")
            pooled.append(po)

            def st1(cc=cc, pr1=pr1):
                nc.vector.tensor_reduce(
                    out=pr1.rearrange("p (y g) -> p y g", g=HP),
                    in_=x_sb[cc].rearrange("p (y g x) -> p y g x",
                                           y=H, g=HP),
                    axis=mybir.AxisListType.X, op=OP.add)

            def st2(pr1=pr1, po=po):
                with nc.allow_low_precision(reason="pool sums fit fp16"):
                    nc.vector.tensor_reduce(
                        out=po.rearrange("p (a b) -> p a b", a=HP),
                        in_=pr1.rearrange("p (hp dy wp) -> p hp wp dy",
                                          hp=HP, dy=WIN),
                        axis=mybir.AxisListType.X, op=OP.add)
            ops.append(st1)
            ops.append(st2)
        return ops, pooled

    def unified(b, x_next):
        """attention + projection for image b fully interleaved with the
        qkv/gq front-end and dw/gating chain of image b+1."""
        par = b % 2
        npar = 1 - par
        cat_hi = cat_hi2[par]
        cat_lo = cat_lo2[par]
        gq_sb = gq_sb2[par]
        nz_q, nz_k, nz_v = zbufs[npar]
        ncat_hi = cat_hi2[npar]
        qk_t = {}
        hs = {}
        have = x_next is not None
        if have:
            pops, pooled = pool_closures(x_next)
        for i in range(NT + 5):
            if have and i < NT:
                qkv_tile(b + 1, i, x_next)
                gq_tile(b + 1, i, x_next)
            if i < NT:
                scores_stage(par, i, gq_sb)
            if have and 2 <= i < NT + 2:
                t = i - 2
                pdq = dw_mm8(nz_q, 0, t, "pdq")
                q_t = sm.tile([128, TS], F16, tag="q_t", name="q_t")
                nc.scalar.activation(out=q_t[:], in_=pdq[:],
                                     func=AF.Identity, bias=bias_q)
                pdk = dw_mm8(nz_k, 1, t, "pdk")
                qk = sm.tile([128, TS], F16, tag="qk_t", name="qk_t")
                with nc.allow_low_precision(reason="qk product fits fp16"):
                    nc.vector.scalar_tensor_tensor(
                        out=qk[:], in0=pdk[:], scalar=bias_k, in1=q_t[:],
                        op0=OP.add, op1=OP.mult)
                qk_t[t] = qk
            if 1 <= i <= NT:
                den_stage(i - 1)
            if have and 3 <= i < NT + 3:
                t = i - 3
                pa1 = ps.tile([128, TS], F32, tag="py", name="pa1")
                nc.tensor.matmul(pa1[:], wact1[:], qk_t[t][:],
                                 start=True, stop=True)
                t_a = sm.tile([128, TS], F16, tag="t_a", name="t_a")
                nc.scalar.activation(out=t_a[:], in_=pa1[:],
                                     func=AF.Identity, bias=bact1)
                u_t = sm.tile([128, TS], F16, tag="u_t", name="u_t")
                nc.scalar.activation(out=u_t[:], in_=t_a[:],
                                     func=AF.Relu, bias=const3)
                h_t = sm.tile([128, TS], F16, tag="hs_t", name="hs_t")
                with nc.allow_low_precision(reason="hardswish fits fp16"):
                    nc.vector.scalar_tensor_tensor(
                        out=h_t[:], in0=u_t[:], scalar=6.0, in1=t_a[:],
                        op0=OP.min, op1=OP.mult)
                hs[t] = h_t
            if 2 <= i <= NT + 1:
                av_stage(i - 2, par, cat_lo)
            if have and 4 <= i < NT + 4:
                t = i - 4
                sl = slice(t * TS, (t + 1) * TS)
                pa2 = ps.tile([128, TS], F32, tag="py", name="pa2")
                nc.tensor.matmul(pa2[:], wact2[:], hs[t][:],
                                 start=True, stop=True)
                g_t = sm.tile([128, TS], F16, tag="g_t", name="g_t")
                nc.scalar.activation(out=g_t[:], in_=pa2[:], func=AF.Tanh,
                                     bias=bact2)
                if t in OFFV:
                    acc = dwv_vector(nz_v, t)
                    with nc.allow_low_precision(reason="gated out fp16"):
                        nc.vector.scalar_tensor_tensor(
                            out=ncat_hi[:, sl], in0=acc[:], scalar=1.0,
                            in1=g_t[:], op0=OP.mult, op1=OP.mult)
                else:
                    pdv = dw_mm(nz_v, 2, t, "pdv")
                    with nc.allow_low_precision(reason="gated out fp16"):
                        nc.vector.scalar_tensor_tensor(
                            out=ncat_hi[:, sl], in0=pdv[:], scalar=bias_v,
                            in1=g_t[:], op0=OP.add, op1=OP.mult)
            if 3 <= i <= NT + 2:
                proj_stage(b, i - 3, cat_hi, cat_lo)
            if have and 2 <= i <= 5:
                pops[i - 2]()
        if have:
            pool_finish(b + 1, pooled)

    # ---------------- program ----------------
    x_cur = load_x(0)
    pooled = pool_reduce(x_cur)
    for t in range(NT):
        qkv_tile(0, t, x_cur)
        gq_tile(0, t, x_cur)
    pool_finish(0, pooled)
    loop_a(0)

    for b in range(n_img):
        x_next = load_x(b + 1) if b + 1 < n_img else None
        unified(b, x_next)


def _build(n_img=B):
    nc = bacc.Bacc("TRN2", target_bir_lowering=False, debug=False,
                   num_devices=N_CORES)
    dt = nc.dram_tensor
    d = {
        "x": dt("x", [B, 2, 128, HW], F16, kind="ExternalInput").ap(),
        "x8": dt("x8", [B, 128, 2 * HW], F8, kind="ExternalInput").ap(),
        "wf16": dt("wf16", [128, W16_COLS], F16, kind="ExternalInput").ap(),
        "wf32": dt("wf32", [128, W32_COLS], F32, kind="ExternalInput").ap(),
        "wf8": dt("wf8", [128, W8_COLS], F8, kind="ExternalInput").ap(),
        "out": dt("out", [B, 2, 128, HW], F16, kind="ExternalOutput").ap(),
    }
    with tile.TileContext(nc) as tc, ExitStack() as ctx:
        _body(ctx, tc, d, n_img=n_img)
    nc.compile()
    return nc


_NC = None


def _prep_weights(qkv_w, dw_w, dw_b, act1_w, act1_b, act2_w, act2_b,
                  gq_w, gkv_w, proj_w):
    f32 = np.float32
    f16 = np.float16
    sc = np.float32(HEAD_DIM ** -0.5)

    wqkv = qkv_w.T.reshape(2, 128, 384).astype(f16)
    taps = dw_w.reshape(384, 9)            # [c, tap]
    idx = np.arange(128)
    # f16 diag blocks for the v branch only
    dwd = np.zeros((9, 128, 128), dtype=f16)
    for tp in range(9):
        dwd[tp, idx, idx] = taps[256:384, tp]
    dwdiag = dwd.transpose(1, 0, 2).reshape(128, 9 * 128)
    # fp8 DoubleRow pair blocks for q and k branches
    f8 = np.dtype(np.float32)  # placeholder; real cast below
    import ml_dtypes
    e4 = ml_dtypes.float8_e4m3
    wf8 = np.zeros((128, W8_COLS), dtype=e4)
    for br in range(2):
        tb = taps[128 * br:128 * (br + 1)]
        for pr, (ta_, tb_) in enumerate(DW_PAIRS):
            blk = np.zeros((128, 2, 128), np.float32)
            blk[idx, 0, idx] = tb[:, ta_]
            blk[idx, 1, idx] = tb[:, tb_]
            wf8[:, W8_DW + br * W8_BRANCH + pr * 256:
                W8_DW + br * W8_BRANCH + (pr + 1) * 256] = (
                blk.reshape(128, 256).astype(e4))
        t8 = np.zeros((128, 128), np.float32)
        t8[idx, idx] = tb[:, 8]
        wf8[:, W8_DW + br * W8_BRANCH + 1024:
            W8_DW + br * W8_BRANCH + 1152] = t8.astype(e4)
    for bi, wsrc in enumerate((qkv_w[0:128], qkv_w[128:256], gq_w)):
        blk = (wsrc.T.reshape(2, 128, 128).transpose(1, 0, 2)
               .reshape(128, 256))
        wf8[:, W8_QKV + bi * 256:W8_QKV + (bi + 1) * 256] = blk.astype(e4)
    wact1 = (act1_w * sc).T.astype(f16)
    wact2 = (act2_w / 6.0).T.astype(f16)
    wgq = gq_w.T.reshape(2, 128, 128).astype(f16)
    wgkv = (gkv_w / 49.0).T.reshape(2, 128, 256).astype(f16)
    wproj = proj_w.T.reshape(2, 128, 256).astype(f16)
    dm = np.zeros((2, 128, 128), dtype=f16)
    for p in range(2):
        for hl in range(2):
            head = 2 * p + hl
            dm[p, 64 * hl:64 * hl + 64, 32 * head:32 * head + 32] = 1.0

    blocks = {"wqkv0": wqkv[0], "wqkv1": wqkv[1], "dwdiag": dwdiag,
              "wact1": wact1, "wact2": wact2, "wgq0": wgq[0],
              "wgq1": wgq[1], "wgkv0": wgkv[0], "wgkv1": wgkv[1],
              "wproj0": wproj[0], "wproj1": wproj[1],
              "denmask0": dm[0], "denmask1": dm[1]}
    wf16 = np.zeros((128, W16_COLS), dtype=f16)
    for nm, (a, b_) in WCOL.items():
        wf16[:, a:b_] = blocks[nm]

    wf32 = np.zeros((128, W32_COLS), dtype=f32)
    wf32[:, 0:3] = dw_b.reshape(3, 128).T
    wf32[:, 3] = act1_b.astype(f32)
    wf32[:, 4] = act2_b.astype(f32)
    wf32[:, 5:14] = taps[256:384].astype(f32)   # dw-v taps for DVE path
    wf32[:, 14] = 3.0

    return {"wf16": np.ascontiguousarray(wf16),
            "wf32": np.ascontiguousarray(wf32),
            "wf8": np.ascontiguousarray(wf8)}


def _make_in_maps(inputs):
    w = _prep_weights(
        inputs["qkv_w"], inputs["dw_w"], inputs["dw_b"],
        inputs["act1_w"], inputs["act1_b"], inputs["act2_w"],
        inputs["act2_b"], inputs["gq_w"], inputs["gkv_w"],
        inputs["proj_w"])
    x = inputs["x"]
    in_maps = []
    for core in range(N_CORES):
        m = dict(w)
        xc = x[core * B:(core + 1) * B]
        m["x"] = np.ascontiguousarray(
            xc.reshape(B, 2, 128, HW).astype(np.float16))
        import ml_dtypes
        m["x8"] = np.ascontiguousarray(
            xc.reshape(B, 2, 128, HW).transpose(0, 2, 1, 3)
            .reshape(B, 128, 2 * HW).astype(ml_dtypes.float8_e4m3))
        in_maps.append(m)
    return in_maps


def kernel(**inputs):
    global _NC
    if _NC is None:
        _NC = _build()
    in_maps = _make_in_maps(inputs)
    res = run_bass_kernel_spmd(_NC, in_maps, core_ids=list(range(N_CORES)))
    out = np.concatenate([r["out"] for r in res.results], axis=0)
    return out.reshape(B_FULL, C, H, W).astype(np.float32)


# revision 42
# speedup vs baseline: 1.0191x; 1.0056x over previous
"""CloAttention Trainium2 Bass kernel.

Full inputs -> data-parallel over batch across 8 NeuronCores (4 images each)
-> full output.  All matmuls run on the PE in fp16 (1 cycle/row); the 3x3
depthwise conv runs as 9 diagonal-matmul accumulations into PSUM.

Schedule: software-pipelined per image.  Loop A runs the depthwise/gating
chain with a 2-tile skew so the PE never waits on the scalar/vector chain;
loop B runs attention + projection for image b interleaved with the qkv/gq
front-end of image b+1.  Pooling runs on the otherwise-idle GPSIMD engine;
a couple of dw-v tiles per image run as shifted multiply-adds on DVE to
shave PE work.  Weights arrive in two consolidated DMAs and dummy matmuls
warm the PE HAM clock-gate during the initial DMA wait.
"""

import numpy as np
from contextlib import ExitStack

import concourse.bacc as bacc
import concourse.bass as bass
import concourse.tile as tile
from concourse import mybir
from concourse.bass_utils import run_bass_kernel_spmd

F32 = mybir.dt.float32
F16 = mybir.dt.float16
F8 = mybir.dt.float8e4
AF = mybir.ActivationFunctionType
OP = mybir.AluOpType
DR = mybir.MatmulPerfMode.DoubleRow

# dw tap pairing for fp8 DoubleRow matmuls: 4 pairs with a constant
# address delta between the two shifted windows, plus tap 8 standalone
DW_PAIRS = ((0, 1), (3, 4), (6, 7), (2, 5))
W8_BRANCH = 4 * 256 + 128      # cols per branch in the fp8 weight block
W8_QKV = 0                     # qkv DR blocks first (DR LDW needs low offs)
W8_DW = 3 * 256                # dw branches after
W8_COLS = W8_DW + 2 * W8_BRANCH

N_CORES = 8
B_FULL = 32
B = B_FULL // N_CORES          # images per core
C = 256
H = W = 56
HW = H * W                     # 3136
PW = H + 2                     # 58 padded
NT = 7                         # pixel tiles per image
TS = HW // NT                  # 448 = 8 rows of 56
RPT = H // NT                  # 8 rows per tile
HEAD_DIM = 32
SCALER = HEAD_DIM ** -0.5
WIN = 7
HP = H // WIN                  # 8
POOL_N = HP * HP               # 64

OFFV = (1, 4)                  # tiles whose dw-v runs on DVE, not PE

# f16 weight block column offsets
WCOL = {}
_off = 0
for _nm, _w in (("wqkv0", 384), ("wqkv1", 384), ("dwdiag", 1152),
                ("wact1", 128), ("wact2", 128), ("wgq0", 128),
                ("wgq1", 128), ("wgkv0", 256), ("wgkv1", 256),
                ("wproj0", 256), ("wproj1", 256), ("denmask0", 128),
                ("denmask1", 128)):
    WCOL[_nm] = (_off, _off + _w)
    _off += _w
W16_COLS = _off                # 6016
W32_COLS = 15                  # dwb q,k,v | bact1 | bact2 | dwv taps 0..8 | 3.0


def _body(ctx, tc, d, n_img=B):
    nc = tc.nc

    # ---------------- persistent weights (2 consolidated DMAs) ----------
    wpool = ctx.enter_context(tc.tile_pool(name="wpool", bufs=1))

    warm_src = wpool.tile([128, 64], F16, tag="warm_src", name="warm_src")
    nc.vector.memset(warm_src, 0.0)

    wf16 = wpool.tile([128, W16_COLS], F16, tag="wf16", name="wf16")
    nc.sync.dma_start(out=wf16, in_=d["wf16"])
    wf32 = wpool.tile([128, W32_COLS], F32, tag="wf32", name="wf32")
    nc.sync.dma_start(out=wf32, in_=d["wf32"])
    wf8 = wpool.tile([128, W8_COLS], F8, tag="wf8", name="wf8")
    nc.sync.dma_start(out=wf8, in_=d["wf8"])

    def wv(name):
        a, b_ = WCOL[name]
        return wf16[:, a:b_]

    wqkv = [wv("wqkv0"), wv("wqkv1")]
    dwdiag = wv("dwdiag")
    wact1 = wv("wact1")
    wact2 = wv("wact2")
    wgq = [wv("wgq0"), wv("wgq1")]
    wgkv = [wv("wgkv0"), wv("wgkv1")]
    wproj = [wv("wproj0"), wv("wproj1")]
    denmask = [wv("denmask0"), wv("denmask1")]
    bias_q = wf32[:, 0:1]
    bias_k = wf32[:, 1:2]
    bias_v = wf32[:, 2:3]
    bact1 = wf32[:, 3:4]
    bact2 = wf32[:, 4:5]
    const3 = wf32[:, 14:15]

    def w8dr(idx):
        """fp8 DoubleRow lhsT [128, 2, 128] for qkv q(0) / k(1) / gq(2)."""
        off = W8_QKV + idx * 256
        return wf8[:, off:off + 256].rearrange("p (i m) -> p i m", i=2)

    def wv_tap(tap):
        return wf32[:, 5 + tap:6 + tap]

    def dw_lhsT(cc, tap):
        return dwdiag[:, tap * 128:(tap + 1) * 128]    # v branch only

    # padded z buffers, x2 for image parity (borders stay zero; interiors
    # rewritten per image).  q/k are fp8 (read only by the DoubleRow dw
    # matmuls); v stays fp16.
    zbufs = []
    for par in range(2):
        zs = [wpool.tile([128, PW * PW], F8 if j < 2 else F16,
                         tag=f"z{j}_{par}", name=f"z{j}_{par}")
              for j in range(3)]
        for z in zs:
            zg = z.rearrange("p (r c) -> p r c", c=PW)
            nc.vector.memset(zg[:, 0, :], 0.0)          # top border row
            nc.vector.memset(zg[:, PW - 1, :], 0.0)     # bottom border row
            nc.vector.memset(zg[:, :, 0], 0.0)          # left border col
            nc.vector.memset(zg[:, :, PW - 1], 0.0)     # right border col
        zbufs.append(zs)

    # block-diagonal gk (2 heads per matmul at K=128) and zero-padded AV
    # lhsT blocks, x2 parity; zero regions never rewritten -> memset once
    gk2 = []
    av_lhs = []
    for par in range(2):
        g = [wpool.tile([128, 128], F16, tag=f"gk2_{p}_{par}",
                        name=f"gk2_{p}_{par}") for p in range(2)]
        a = [wpool.tile([128, 128], F16, tag=f"av_{p}_{par}",
                        name=f"av_{p}_{par}") for p in range(2)]
        for tbuf in (*g, *a):
            nc.vector.memset(tbuf, 0.0)
        gk2.append(g)
        av_lhs.append(a)

    # ---------------- pools ----------------
    ps = ctx.enter_context(tc.tile_pool(name="ps", bufs=4, space="PSUM"))
    xpool = ctx.enter_context(tc.tile_pool(name="xpool", bufs=3))
    big = ctx.enter_context(tc.tile_pool(name="big", bufs=1))
    sm = ctx.enter_context(tc.tile_pool(name="sm", bufs=3))
    tiny = ctx.enter_context(tc.tile_pool(name="tiny", bufs=2))

    gq_sb2 = [big.tile([128, HW], F16, tag=f"gq_sb{i}", name=f"gq_sb{i}")
              for i in range(2)]
    exp_sb = [big.tile([128, HW], F16, tag=f"exp{p}", name=f"exp{p}")
              for p in range(2)]
    rec_rep = big.tile([128, HW], F32, tag="rec_rep")
    cat_hi2 = [big.tile([128, HW], F16, tag=f"cat_hi{i}", name=f"cat_hi{i}")
               for i in range(2)]
    cat_lo2 = [big.tile([128, HW], F16, tag=f"cat_lo{i}", name=f"cat_lo{i}")
               for i in range(2)]

    zgrid = {id(z): z.rearrange("p (r c) -> p r c", c=PW)
             for zs in zbufs for z in zs}

    def zwin(z, t, dy, dx):
        r0 = RPT * t + dy
        return zgrid[id(z)][:, r0:r0 + RPT, dx:dx + W]

    def zint(z, t):
        r0 = RPT * t + 1
        return zgrid[id(z)][:, r0:r0 + RPT, 1:1 + W]

    # PE warmup: dummy matmuls keep the HAM clock-gate busy while the
    # weight/x DMAs land, so real matmuls start at 2.4 GHz
    for wi in range(48):
        pw = ps.tile([64, 64], F32, tag="py", name="pwarm")
        nc.tensor.matmul(pw[:], warm_src[:], warm_src[:],
                         start=True, stop=True)

    # ---------------- stage helpers ----------------
    def load_x(b):
        x_sb = [xpool.tile([128, HW], F16, tag=f"x{cc}", name=f"x{cc}")
                for cc in range(2)]
        for cc in range(2):
            nc.sync.dma_start(out=x_sb[cc], in_=d["x"][b, cc])
        x8 = xpool.tile([128, 2 * HW], F8, tag="x8", name="x8")
        nc.sync.dma_start(out=x8, in_=d["x8"][b])
        # [p, 2, rows, 56]: the interleave-of-2 must be the third free dim
        # from the innermost (DoubleRow ISA requires n_elem[2]==2)
        x_sb.append(x8.rearrange("p (i r c) -> p i r c", i=2, c=W))
        return x_sb

    def qkv_tile(b, t, x_sb):
        z_q, z_k, z_v = zbufs[b % 2]
        rhs8 = x_sb[2][:, :, t * RPT:(t + 1) * RPT, :]
        for j, (z, eng) in enumerate(
                ((z_q, "act"), (z_k, "act"), (z_v, "dve"))):
            pq = ps.tile([128, TS], F32, tag="py", name="pq")
            if j < 2:
                nc.tensor.matmul(pq[:], w8dr(j), rhs8,
                                 start=True, stop=True, perf_mode=DR)
            else:
                for cc in range(2):
                    nc.tensor.matmul(
                        pq[:], wqkv[cc][:, 256:384],
                        x_sb[cc][:, t * TS:(t + 1) * TS],
                        start=(cc == 0), stop=(cc == 1))
            if eng == "act":
                nc.scalar.copy(out=zint(z, t), in_=pq[:])
            else:
                nc.vector.tensor_copy(out=zint(z, t), in_=pq[:])

    def gq_tile(b, t, x_sb):
        pg = ps.tile([128, TS], F32, tag="py", name="pg")
        for cc in range(2):
            nc.tensor.matmul(pg[:], wgq[cc][:],
                             x_sb[cc][:, t * TS:(t + 1) * TS],
                             start=(cc == 0), stop=(cc == 1))
        nc.vector.tensor_copy(out=gq_sb2[b % 2][:, t * TS:(t + 1) * TS],
                              in_=pg[:])

    def pool_reduce(x_sb):
        """7x7 window sums; issued early so the results have slack."""
        pooled = []
        for cc in range(2):
            pr1 = sm.tile([128, H * HP], F32, tag="pr1", name="pr1")
            nc.vector.tensor_reduce(
                out=pr1.rearrange("p (y g) -> p y g", g=HP),
                in_=x_sb[cc].rearrange("p (y g x) -> p y g x", y=H, g=HP),
                axis=mybir.AxisListType.X, op=OP.add)
            po = tiny.tile([128, POOL_N], F16, tag="po", name="po")
            with nc.allow_low_precision(reason="pool sums fit fp16"):
                nc.vector.tensor_reduce(
                    out=po.rearrange("p (a b) -> p a b", a=HP),
                    in_=pr1.rearrange("p (hp dy wp) -> p hp wp dy",
                                      hp=HP, dy=WIN),
                    axis=mybir.AxisListType.X, op=OP.add)
            pooled.append(po)
        return pooled

    def pool_finish(b, pooled):
        """global-kv matmuls + lhsT packing for image b's attention."""
        par = b % 2
        pgk = ps.tile([128, POOL_N], F32, tag="py", name="pgk")
        for cc in range(2):
            nc.tensor.matmul(pgk[:], wgkv[cc][:, 0:128], pooled[cc][:],
                             start=(cc == 0), stop=(cc == 1))
        for p in range(2):
            for hl in range(2):
                h = 2 * p + hl
                nc.scalar.copy(
                    out=gk2[par][p][32 * h:32 * h + 32,
                                    64 * hl:64 * hl + 64],
                    in_=pgk[32 * h:32 * h + 32, :])
        pgv = ps.tile([POOL_N, 128], F32, tag="py", name="pgv")
        for cc in range(2):
            nc.tensor.matmul(pgv[:], pooled[cc][:], wgkv[cc][:, 128:256],
                             start=(cc == 0), stop=(cc == 1))
        gvT = tiny.tile([POOL_N, 128], F16, tag="gvT", name="gvT")
        nc.scalar.copy(out=gvT[:], in_=pgv[:])
        av0, av1 = av_lhs[par]
        nc.vector.tensor_copy(out=av0[0:64, 0:32], in_=gvT[:, 0:32])
        nc.sync.dma_start(out=av0[64:128, 32:64], in_=gvT[:, 32:64])
        nc.vector.tensor_copy(out=av1[0:64, 64:96], in_=gvT[:, 64:96])
        nc.sync.dma_start(out=av1[64:128, 96:128], in_=gvT[:, 96:128])

    def dw_mm(z, cc, t, psname):
        p = ps.tile([128, TS], F32, tag="px", name=psname)
        for tap in range(9):
            dy, dx = divmod(tap, 3)
            nc.tensor.matmul(p[:], dw_lhsT(cc, tap), zwin(z, t, dy, dx),
                             start=(tap == 0), stop=(tap == 8))
        return p

    def dw_mm8(z8, br, t, psname):
        """dw conv via 4 fp8 DoubleRow pair-matmuls + 1 plain fp8 matmul."""
        p = ps.tile([128, TS], F32, tag="px", name=psname)
        zg = zgrid[id(z8)]
        for pr, (tapA, tapB) in enumerate(DW_PAIRS):
            dyA, dxA = divmod(tapA, 3)
            dyB, dxB = divmod(tapB, 3)
            delta = (dyB - dyA) * PW + (dxB - dxA)
            w = zg[:, RPT * t + dyA:RPT * t + dyA + RPT, dxA:dxA + W]
            pa = list(w.ap)
            rhs = bass.AP(w.tensor, w.offset,
                          [pa[0], [delta, 2], pa[1], pa[2]])
            lhsT = wf8[:, W8_DW + br * W8_BRANCH + pr * 256:
                       W8_DW + br * W8_BRANCH + (pr + 1) * 256]
            nc.tensor.matmul(p[:], lhsT.rearrange("p (i m) -> p i m", i=2),
                             rhs, start=(pr == 0), stop=False,
                             perf_mode=mybir.MatmulPerfMode.DoubleRow)
        nc.tensor.matmul(p[:],
                         wf8[:, W8_DW + br * W8_BRANCH + 1024:
                             W8_DW + br * W8_BRANCH + 1152],
                         zwin(z8, t, 2, 2), start=False, stop=True)
        return p

    def dwv_vector(z_v, t):
        """dw-v for one tile as 9 shifted multiply-adds on DVE; returns
        the accumulated (dwv + bias_v) tile in fp16."""
        acc = sm.tile([128, TS], F16, tag="accv", name="accv")
        with nc.allow_low_precision(reason="dwv fits fp16"):
            nc.vector.tensor_scalar(
                out=acc[:], in0=zwin(z_v, t, 0, 0), scalar1=wv_tap(0),
                scalar2=bias_v, op0=OP.mult, op1=OP.add)
            for tap in range(1, 9):
                dy, dx = divmod(tap, 3)
                nacc = sm.tile([128, TS], F16, tag="accv", name="accv")
                nc.vector.scalar_tensor_tensor(
                    out=nacc[:], in0=zwin(z_v, t, dy, dx),
                    scalar=wv_tap(tap), in1=acc[:],
                    op0=OP.mult, op1=OP.add)
                acc = nacc
        return acc

    # ---------------- pipelined loops ----------------
    def loop_a(b):
        """dwconv + gating chain, 2-tile skew."""
        z_q, z_k, z_v = zbufs[b % 2]
        cat_hi = cat_hi2[b % 2]
        qk_t = {}
        hs = {}
        for i in range(NT + 2):
            if i < NT:
                t = i
                pdq = dw_mm8(z_q, 0, t, "pdq")
                q_t = sm.tile([128, TS], F16, tag="q_t", name="q_t")
                nc.scalar.activation(out=q_t[:], in_=pdq[:],
                                     func=AF.Identity, bias=bias_q)
                pdk = dw_mm8(z_k, 1, t, "pdk")
                qk = sm.tile([128, TS], F16, tag="qk_t", name="qk_t")
                with nc.allow_low_precision(reason="qk product fits fp16"):
                    nc.vector.scalar_tensor_tensor(
                        out=qk[:], in0=pdk[:], scalar=bias_k, in1=q_t[:],
                        op0=OP.add, op1=OP.mult)
                qk_t[t] = qk
            if 1 <= i <= NT:
                t = i - 1
                pa1 = ps.tile([128, TS], F32, tag="py", name="pa1")
                nc.tensor.matmul(pa1[:], wact1[:], qk_t[t][:],
                                 start=True, stop=True)
                t_a = sm.tile([128, TS], F16, tag="t_a", name="t_a")
                nc.scalar.activation(out=t_a[:], in_=pa1[:],
                                     func=AF.Identity, bias=bact1)
                u_t = sm.tile([128, TS], F16, tag="u_t", name="u_t")
                nc.scalar.activation(out=u_t[:], in_=t_a[:],
                                     func=AF.Relu, bias=const3)
                h_t = sm.tile([128, TS], F16, tag="hs_t", name="hs_t")
                with nc.allow_low_precision(reason="hardswish fits fp16"):
                    nc.vector.scalar_tensor_tensor(
                        out=h_t[:], in0=u_t[:], scalar=6.0, in1=t_a[:],
                        op0=OP.min, op1=OP.mult)
                hs[t] = h_t
            if 2 <= i:
                t = i - 2
                sl = slice(t * TS, (t + 1) * TS)
                pa2 = ps.tile([128, TS], F32, tag="py", name="pa2")
                nc.tensor.matmul(pa2[:], wact2[:], hs[t][:],
                                 start=True, stop=True)
                g_t = sm.tile([128, TS], F16, tag="g_t", name="g_t")
                nc.scalar.activation(out=g_t[:], in_=pa2[:], func=AF.Tanh,
                                     bias=bact2)
                if t in OFFV:
                    acc = dwv_vector(z_v, t)
                    with nc.allow_low_precision(reason="gated out fp16"):
                        nc.vector.scalar_tensor_tensor(
                            out=cat_hi[:, sl], in0=acc[:], scalar=1.0,
                            in1=g_t[:], op0=OP.mult, op1=OP.mult)
                else:
                    pdv = dw_mm(z_v, 2, t, "pdv")
                    with nc.allow_low_precision(reason="gated out fp16"):
                        nc.vector.scalar_tensor_tensor(
                            out=cat_hi[:, sl], in0=pdv[:], scalar=bias_v,
                            in1=g_t[:], op0=OP.add, op1=OP.mult)

    def scores_stage(par, t, gq_sb):
        sl = slice(t * TS, (t + 1) * TS)
        for p in range(2):
            pat = ps.tile([128, TS], F32, tag="px", name="pat")
            nc.tensor.matmul(pat[:], gk2[par][p][:], gq_sb[:, sl],
                             start=True, stop=True)
            nc.scalar.activation(out=exp_sb[p][:, sl], in_=pat[:],
                                 func=AF.Exp, scale=float(SCALER))

    def den_stage(t):
        sl = slice(t * TS, (t + 1) * TS)
        pden = ps.tile([128, TS], F32, tag="px", name="pden")
        for p in range(2):
            nc.tensor.matmul(pden[:], denmask[p][:], exp_sb[p][:, sl],
                             start=(p == 0), stop=(p == 1))
        nc.vector.reciprocal_approx_fast(out=rec_rep[:, sl], in_=pden[:])

    def av_stage(t, par, cat_lo):
        sl = slice(t * TS, (t + 1) * TS)
        av0, av1 = av_lhs[par]
        pav = ps.tile([128, TS], F32, tag="px", name="pav")
        nc.tensor.matmul(pav[:], av0[:], exp_sb[0][:, sl],
                         start=True, stop=False)
        nc.tensor.matmul(pav[:], av1[:], exp_sb[1][:, sl],
                         start=False, stop=True)
        with nc.allow_low_precision(reason="attn out fits fp16"):
            nc.vector.scalar_tensor_tensor(
                out=cat_lo[:, sl], in0=pav[:], scalar=1.0,
                in1=rec_rep[:, sl], op0=OP.mult, op1=OP.mult)

    def proj_stage(b, t, cat_hi, cat_lo):
        sl = slice(t * TS, (t + 1) * TS)
        for m in range(2):
            pp = ps.tile([128, TS], F32, tag="py", name="pp")
            nc.tensor.matmul(pp[:], wproj[0][:, m * 128:(m + 1) * 128],
                             cat_hi[:, sl], start=True, stop=False)
            nc.tensor.matmul(pp[:], wproj[1][:, m * 128:(m + 1) * 128],
                             cat_lo[:, sl], start=False, stop=True)
            o_t = sm.tile([128, TS], F16, tag=f"o_t{m}", name=f"o_t{m}")
            if m == 0:
                nc.scalar.copy(out=o_t[:], in_=pp[:])
            else:
                nc.vector.tensor_copy(out=o_t[:], in_=pp[:])
            nc.sync.dma_start(out=d["out"][b, m, :, sl], in_=o_t)

    def pool_closures(x_sb):
        """4 GPSIMD-free pooling ops as closures, sprinkled across stages."""
        ops = []
        pooled = []
        for cc in range(2):
            pr1 = sm.tile([128, H * HP], F32, tag="pr1", name="pr1")
            po = tiny.tile([128, POOL_N], F16, tag="po", name="po")
            pooled.append(po)

            def st1(cc=cc, pr1=pr1):
                nc.vector.tensor_reduce(
                    out=pr1.rearrange("p (y g) -> p y g", g=HP),
                    in_=x_sb[cc].rearrange("p (y g x) -> p y g x",
                                           y=H, g=HP),
                    axis=mybir.AxisListType.X, op=OP.add)

            def st2(pr1=pr1, po=po):
                with nc.allow_low_precision(reason="pool sums fit fp16"):
                    nc.vector.tensor_reduce(
                        out=po.rearrange("p (a b) -> p a b", a=HP),
                        in_=pr1.rearrange("p (hp dy wp) -> p hp wp dy",
                                          hp=HP, dy=WIN),
                        axis=mybir.AxisListType.X, op=OP.add)
            ops.append(st1)
            ops.append(st2)
        return ops, pooled

    def unified(b, x_next):
        """attention + projection for image b fully interleaved with the
        qkv/gq front-end and dw/gating chain of image b+1."""
        par = b % 2
        npar = 1 - par
        cat_hi = cat_hi2[par]
        cat_lo = cat_lo2[par]
        gq_sb = gq_sb2[par]
        nz_q, nz_k, nz_v = zbufs[npar]
        ncat_hi = cat_hi2[npar]
        qk_t = {}
        hs = {}
        have = x_next is not None
        if have:
            pops, pooled = pool_closures(x_next)
        for i in range(NT + 5):
            if have and i < NT:
                qkv_tile(b + 1, i, x_next)
                gq_tile(b + 1, i, x_next)
            if i < NT:
                scores_stage(par, i, gq_sb)
            if have and 2 <= i < NT + 2:
                t = i - 2
                pdq = dw_mm8(nz_q, 0, t, "pdq")
                q_t = sm.tile([128, TS], F16, tag="q_t", name="q_t")
                nc.scalar.activation(out=q_t[:], in_=pdq[:],
                                     func=AF.Identity, bias=bias_q)
                pdk = dw_mm8(nz_k, 1, t, "pdk")
                qk = sm.tile([128, TS], F16, tag="qk_t", name="qk_t")
                with nc.allow_low_precision(reason="qk product fits fp16"):
                    nc.vector.scalar_tensor_tensor(
                        out=qk[:], in0=pdk[:], scalar=bias_k, in1=q_t[:],
                        op0=OP.add, op1=OP.mult)
                qk_t[t] = qk
            if 1 <= i <= NT:
                den_stage(i - 1)
            if have and 3 <= i < NT + 3:
                t = i - 3
                pa1 = ps.tile([128, TS], F32, tag="py", name="pa1")
                nc.tensor.matmul(pa1[:], wact1[:], qk_t[t][:],
                                 start=True, stop=True)
                t_a = sm.tile([128, TS], F16, tag="t_a", name="t_a")
                nc.scalar.activation(out=t_a[:], in_=pa1[:],
                                     func=AF.Identity, bias=bact1)
                u_t = sm.tile([128, TS], F16, tag="u_t", name="u_t")
                nc.scalar.activation(out=u_t[:], in_=t_a[:],
                                     func=AF.Relu, bias=const3)
                h_t = sm.tile([128, TS], F16, tag="hs_t", name="hs_t")
                with nc.allow_low_precision(reason="hardswish fits fp16"):
                    nc.vector.scalar_tensor_tensor(
                        out=h_t[:], in0=u_t[:], scalar=6.0, in1=t_a[:],
                        op0=OP.min, op1=OP.mult)
                hs[t] = h_t
            if 2 <= i <= NT + 1:
                av_stage(i - 2, par, cat_lo)
            if have and 4 <= i < NT + 4:
                t = i - 4
                sl = slice(t * TS, (t + 1) * TS)
                pa2 = ps.tile([128, TS], F32, tag="py", name="pa2")
                nc.tensor.matmul(pa2[:], wact2[:], hs[t][:],
                                 start=True, stop=True)
                g_t = sm.tile([128, TS], F16, tag="g_t", name="g_t")
                nc.scalar.activation(out=g_t[:], in_=pa2[:], func=AF.Tanh,
                                     bias=bact2)
                if t in OFFV:
                    acc = dwv_vector(nz_v, t)
                    with nc.allow_low_precision(reason="gated out fp16"):
                        nc.vector.scalar_tensor_tensor(
                            out=ncat_hi[:, sl], in0=acc[:], scalar=1.0,
                            in1=g_t[:], op0=OP.mult, op1=OP.mult)
                else:
                    pdv = dw_mm(nz_v, 2, t, "pdv")
                    with nc.allow_low_precision(reason="gated out fp16"):
                        nc.vector.scalar_tensor_tensor(
                            out=ncat_hi[:, sl], in0=pdv[:], scalar=bias_v,
                            in1=g_t[:], op0=OP.add, op1=OP.mult)
            if 3 <= i <= NT + 2:
                proj_stage(b, i - 3, cat_hi, cat_lo)
            if have and 2 <= i <= 5:
                pops[i - 2]()
        if have:
            pool_finish(b + 1, pooled)

    # ---------------- program ----------------
    x_cur = load_x(0)
    pooled = pool_reduce(x_cur)
    for t in range(NT):
        qkv_tile(0, t, x_cur)
        gq_tile(0, t, x_cur)
    pool_finish(0, pooled)
    # prefetch x two images ahead: each DMA gets a full iteration of lead
    # so the stage-0 qkv of the next image never waits on it
    nxt = load_x(1) if n_img > 1 else None
    loop_a(0)

    for b in range(n_img):
        cur_next = nxt
        nxt = load_x(b + 2) if b + 2 < n_img else None
        unified(b, cur_next)


def _build(n_img=B):
    nc = bacc.Bacc("TRN2", target_bir_lowering=False, debug=False,
                   num_devices=N_CORES)
    dt = nc.dram_tensor
    d = {
        "x": dt("x", [B, 2, 128, HW], F16, kind="ExternalInput").ap(),
        "x8": dt("x8", [B, 128, 2 * HW], F8, kind="ExternalInput").ap(),
        "wf16": dt("wf16", [128, W16_COLS], F16, kind="ExternalInput").ap(),
        "wf32": dt("wf32", [128, W32_COLS], F32, kind="ExternalInput").ap(),
        "wf8": dt("wf8", [128, W8_COLS], F8, kind="ExternalInput").ap(),
        "out": dt("out", [B, 2, 128, HW], F16, kind="ExternalOutput").ap(),
    }
    with tile.TileContext(nc) as tc, ExitStack() as ctx:
        _body(ctx, tc, d, n_img=n_img)
    nc.compile()
    return nc


_NC = None


def _prep_weights(qkv_w, dw_w, dw_b, act1_w, act1_b, act2_w, act2_b,
                  gq_w, gkv_w, proj_w):
    f32 = np.float32
    f16 = np.float16
    sc = np.float32(HEAD_DIM ** -0.5)

    wqkv = qkv_w.T.reshape(2, 128, 384).astype(f16)
    taps = dw_w.reshape(384, 9)            # [c, tap]
    idx = np.arange(128)
    # f16 diag blocks for the v branch only
    dwd = np.zeros((9, 128, 128), dtype=f16)
    for tp in range(9):
        dwd[tp, idx, idx] = taps[256:384, tp]
    dwdiag = dwd.transpose(1, 0, 2).reshape(128, 9 * 128)
    # fp8 DoubleRow pair blocks for q and k branches
    f8 = np.dtype(np.float32)  # placeholder; real cast below
    import ml_dtypes
    e4 = ml_dtypes.float8_e4m3
    wf8 = np.zeros((128, W8_COLS), dtype=e4)
    for br in range(2):
        tb = taps[128 * br:128 * (br + 1)]
        for pr, (ta_, tb_) in enumerate(DW_PAIRS):
            blk = np.zeros((128, 2, 128), np.float32)
            blk[idx, 0, idx] = tb[:, ta_]
            blk[idx, 1, idx] = tb[:, tb_]
            wf8[:, W8_DW + br * W8_BRANCH + pr * 256:
                W8_DW + br * W8_BRANCH + (pr + 1) * 256] = (
                blk.reshape(128, 256).astype(e4))
        t8 = np.zeros((128, 128), np.float32)
        t8[idx, idx] = tb[:, 8]
        wf8[:, W8_DW + br * W8_BRANCH + 1024:
            W8_DW + br * W8_BRANCH + 1152] = t8.astype(e4)
    for bi, wsrc in enumerate((qkv_w[0:128], qkv_w[128:256], gq_w)):
        blk = (wsrc.T.reshape(2, 128, 128).transpose(1, 0, 2)
               .reshape(128, 256))
        wf8[:, W8_QKV + bi * 256:W8_QKV + (bi + 1) * 256] = blk.astype(e4)
    wact1 = (act1_w * sc).T.astype(f16)
    wact2 = (act2_w / 6.0).T.astype(f16)
    wgq = gq_w.T.reshape(2, 128, 128).astype(f16)
    wgkv = (gkv_w / 49.0).T.reshape(2, 128, 256).astype(f16)
    wproj = proj_w.T.reshape(2, 128, 256).astype(f16)
    dm = np.zeros((2, 128, 128), dtype=f16)
    for p in range(2):
        for hl in range(2):
            head = 2 * p + hl
            dm[p, 64 * hl:64 * hl + 64, 32 * head:32 * head + 32] = 1.0

    blocks = {"wqkv0": wqkv[0], "wqkv1": wqkv[1], "dwdiag": dwdiag,
              "wact1": wact1, "wact2": wact2, "wgq0": wgq[0],
              "wgq1": wgq[1], "wgkv0": wgkv[0], "wgkv1": wgkv[1],
              "wproj0": wproj[0], "wproj1": wproj[1],
              "denmask0": dm[0], "denmask1": dm[1]}
    wf16 = np.zeros((128, W16_COLS), dtype=f16)
    for nm, (a, b_) in WCOL.items():
        wf16[:, a:b_] = blocks[nm]

    wf32 = np.zeros((128, W32_COLS), dtype=f32)
    wf32[:, 0:3] = dw_b.reshape(3, 128).T
    wf32[:, 3] = act1_b.astype(f32)
    wf32[:, 4] = act2_b.astype(f32)
    wf32[:, 5:14] = taps[256:384].astype(f32)   # dw-v taps for DVE path
    wf32[:, 14] = 3.0

    return {"wf16": np.ascontiguousarray(wf16),
            "wf32": np.ascontiguousarray(wf32),
            "wf8": np.ascontiguousarray(wf8)}


def _make_in_maps(inputs):
    w = _prep_weights(
        inputs["qkv_w"], inputs["dw_w"], inputs["dw_b"],
        inputs["act1_w"], inputs["act1_b"], inputs["act2_w"],
        inputs["act2_b"], inputs["gq_w"], inputs["gkv_w"],
        inputs["proj_w"])
    x = inputs["x"]
    in_maps = []
    for core in range(N_CORES):
        m = dict(w)
        xc = x[core * B:(core + 1) * B]
        m["x"] = np.ascontiguousarray(
            xc.reshape(B, 2, 128, HW).astype(np.float16))
        import ml_dtypes
        m["x8"] = np.ascontiguousarray(
            xc.reshape(B, 2, 128, HW).transpose(0, 2, 1, 3)
            .reshape(B, 128, 2 * HW).astype(ml_dtypes.float8_e4m3))
        in_maps.append(m)
    return in_maps


def kernel(**inputs):
    global _NC
    if _NC is None:
        _NC = _build()
    in_maps = _make_in_maps(inputs)
    res = run_bass_kernel_spmd(_NC, in_maps, core_ids=list(range(N_CORES)))
    out = np.concatenate([r["out"] for r in res.results], axis=0)
    return out.reshape(B_FULL, C, H, W).astype(np.float32)
